# revision 17
# baseline (speedup 1.0000x reference)
"""Causal RoPE self-attention, distributed over 8 TRN2 NeuronCores.

Sharding: batch (2) x head-groups (4 heads each) -> 8 cores.
Each core computes, for its (batch b, head-group hg):
    q/k/v projections for its 4 heads (tensor-parallel column split),
    RoPE, causal attention, and the row-parallel slice of the output
    projection, producing a partial output partialT = WoS^T @ attnT
    of shape [E, S].  The host sums the 4 partials per batch and adds bo.

On-device layout notes:
  - activations live transposed: qT/kT are [head-dim, seq] so the
    score matmul sT[k, q] = K Q^T contracts over d on partitions, and
    softmax denominators come from an extra all-ones column in V.
  - matmuls run with float32r operand views (full fp32 storage,
    1 cycle/row TensorEngine rate); exp'd scores and V are bf16.
"""

import numpy as np

import concourse.bass as bass
import concourse.tile as tile
from concourse import bacc, mybir
from concourse.bass_utils import run_bass_kernel_spmd

F32 = mybir.dt.float32
F32R = mybir.dt.float32r
BF16 = mybir.dt.bfloat16
AF = mybir.ActivationFunctionType

B, S, E = 2, 2048, 1024
H, D = 16, 64
HPG = 4                # heads per core
DH = HPG * D           # 256 head-dims per core
NE = E // 128          # 8 e-chunks
NST = S // 128         # 16 s-tiles / key blocks
NSL = S // 512         # 4 q-slices
ROPE_BASE = 10000.0
MASK_VAL = -30000.0

_SWAP_MASK = [i ^ 1 for i in range(32)]


def build_nc():
    """Build + compile the per-core Bass graph (same graph on all 8 cores)."""
    nc = bacc.Bacc("TRN2", target_bir_lowering=False, debug=False, num_devices=8)

    def din(name, shape, dt=F32):
        return nc.dram_tensor(name, shape, dt, kind="ExternalInput").ap()

    xT = din("xT", [E, S], F32R)
    wqT = din("wqT", [E, DH], F32R)
    wkT = din("wkT", [E, DH], F32R)
    wvT = din("wvT", [E, DH], F32R)
    woST = din("woST", [DH, E], F32R)
    bq2 = din("bq2", [128, 2])
    bk2 = din("bk2", [128, 2])
    bvbc = din("bvbc", [128, DH])
    cos2 = din("cos2", [128, S])            # cosT duplicated on both halves
    sin2 = din("sin2", [128, S])            # signed sinT duplicated on both halves
    mask2 = din("mask2", [128, 2, 128])     # causal tri mask, duplicated x2
    out = nc.dram_tensor("out", [E, S], F32, kind="ExternalOutput").ap()

    xT_r = xT.rearrange("(n p) s -> n p s", p=128)
    wq_r = wqT.rearrange("(n p) d -> n p d", p=128)
    wk_r = wkT.rearrange("(n p) d -> n p d", p=128)
    wv_r = wvT.rearrange("(n p) d -> n p d", p=128)
    wo_r = woST.rearrange("(n p) e -> n p e", p=128)
    out_r = out.rearrange("(n p) s -> n p s", p=128)

    with tile.TileContext(nc) as tc, nc.allow_low_precision(
            reason="float32r matmul operands (fp32 storage, TF32-rate)"):
        _emit(tc, nc, dict(
            xT_r=xT_r, wq_r=wq_r, wk_r=wk_r, wv_r=wv_r, wo_r=wo_r, out_r=out_r,
            bq2=bq2, bk2=bk2, bvbc=bvbc, cos2=cos2, sin2=sin2, mask2=mask2,
        ))
    nc.compile()
    return nc


def _emit(tc, nc, d):
    from contextlib import ExitStack
    ctx = ExitStack()
    with ctx:
        consts = ctx.enter_context(tc.tile_pool(name="consts", bufs=1))
        px = ctx.enter_context(tc.tile_pool(name="px", bufs=8))
        pwq = ctx.enter_context(tc.tile_pool(name="pwq", bufs=8))
        pwk = ctx.enter_context(tc.tile_pool(name="pwk", bufs=8))
        pwv = ctx.enter_context(tc.tile_pool(name="pwv", bufs=8))
        pwo = ctx.enter_context(tc.tile_pool(name="pwo", bufs=2))
        pqt = ctx.enter_context(tc.tile_pool(name="pqt", bufs=4))
        pkt = ctx.enter_context(tc.tile_pool(name="pkt", bufs=4))
        pv = ctx.enter_context(tc.tile_pool(name="pv", bufs=16))
        pat = ctx.enter_context(tc.tile_pool(name="pat", bufs=4))
        ptmp = ctx.enter_context(tc.tile_pool(name="ptmp", bufs=4))
        pe_ = ctx.enter_context(tc.tile_pool(name="pe", bufs=3))
        prec = ctx.enter_context(tc.tile_pool(name="prec", bufs=4))
        pdram = ctx.enter_context(tc.tile_pool(name="pdram", bufs=4, space="DRAM"))
        psc = ctx.enter_context(tc.tile_pool(name="psc", bufs=2, space="PSUM"))
        ppv = ctx.enter_context(tc.tile_pool(name="ppv", bufs=2, space="PSUM"))

        # ---- constants ----
        cos2_sb = consts.tile([128, S], F32)
        nc.sync.dma_start(cos2_sb, d["cos2"])
        sin2_sb = consts.tile([128, S], F32)
        nc.sync.dma_start(sin2_sb, d["sin2"])
        mask_sb = consts.tile([128, 2, 128], F32)
        nc.sync.dma_start(mask_sb, d["mask2"])
        bq2_sb = consts.tile([128, 2], F32)
        nc.sync.dma_start(bq2_sb, d["bq2"])
        bk2_sb = consts.tile([128, 2], F32)
        nc.sync.dma_start(bk2_sb, d["bk2"])
        bvbc_sb = consts.tile([128, DH], F32)
        nc.sync.dma_start(bvbc_sb, d["bvbc"])

        # ---- weights ----
        wq_sb, wk_sb, wv_sb = [], [], []
        for e in range(NE):
            t = pwq.tile([128, DH], F32R, tag="wq")
            nc.sync.dma_start(t, d["wq_r"][e])
            wq_sb.append(t)
            t = pwk.tile([128, DH], F32R, tag="wk")
            nc.sync.dma_start(t, d["wk_r"][e])
            wk_sb.append(t)
            t = pwv.tile([128, DH], F32R, tag="wv")
            nc.sync.dma_start(t, d["wv_r"][e])
            wv_sb.append(t)
        wo_sb = []
        for p in range(2):
            t = pwo.tile([128, E], F32R, tag="wo")
            nc.sync.dma_start(t, d["wo_r"][p])
            wo_sb.append(t)

        # ---- x^T ----
        xt_sb = []
        for e in range(NE):
            t = px.tile([128, S], F32R, tag="xt")
            nc.sync.dma_start(t, d["xT_r"][e])
            xt_sb.append(t)

        # ---- q/k projections + RoPE ----
        def qk_proj(w_sb, bias_sb, dst_pool, dst_tag):
            tiles = {}
            for p in range(2):
                for sp in range(2):   # slice-pairs of 1024 cols
                    ps = psc.tile([128, 2, 512], F32, tag="sc")
                    for half in range(2):
                        scol = (sp * 2 + half) * 512
                        for e in range(NE):
                            nc.tensor.matmul(
                                ps[:, half, :],
                                w_sb[e][:, p * 128:(p + 1) * 128],
                                xt_sb[e][:, scol:scol + 512],
                                start=(e == 0), stop=(e == NE - 1),
                            )
                    cols = slice(sp * 1024, (sp + 1) * 1024)
                    tq = ptmp.tile([128, 1024], F32, tag="tmp")
                    nc.scalar.activation(
                        tq.rearrange("p (a b) -> p a b", b=512), ps,
                        AF.Identity, bias=bias_sb[:, p:p + 1])
                    tsh = ptmp.tile([128, 1024], F32, tag="tmp")
                    nc.vector.stream_shuffle(tsh, tq, _SWAP_MASK)
                    nc.vector.tensor_mul(tsh, tsh, sin2_sb[:, cols])
                    nc.vector.tensor_mul(tq, tq, cos2_sb[:, cols])
                    qt = dst_pool.tile([128, 1024], F32R, tag=dst_tag)
                    nc.vector.tensor_add(qt, tq, tsh)
                    tiles[(p, sp)] = qt
            return tiles

        qt_tiles = qk_proj(wq_sb, bq2_sb, pqt, "qt")
        kt_tiles = qk_proj(wk_sb, bk2_sb, pkt, "kt")

        # ---- v projection (natural [s, d] layout, bf16, ones col per head) ----
        v_sb = []
        for st in range(NST):
            psv = psc.tile([128, DH], F32, tag="sc")
            for e in range(NE):
                nc.tensor.matmul(
                    psv,
                    xt_sb[e][:, st * 128:(st + 1) * 128],
                    wv_sb[e],
                    start=(e == 0), stop=(e == NE - 1),
                )
            vt = pv.tile([128, HPG, 65], BF16, tag="v")
            nc.vector.memset(vt[:, :, 64:65], 1.0)
            nc.vector.tensor_add(
                vt[:, :, 0:64],
                psv.rearrange("p (h dd) -> p h dd", dd=64),
                bvbc_sb.rearrange("p (h dd) -> p h dd", dd=64),
            )
            v_sb.append(vt)

        # ---- attention + output projection, per q-slice ----
        at_tiles = {}
        for j in range(NSL):
            for p in range(2):
                pvA = ppv.tile([128, 512], F32, tag="ppv")
                pvB = ppv.tile([128, 512], F32, tag="ppv")
                nkb = 4 * j + 4
                for kb in range(nkb):
                    m = kb - 4 * j
                    c0 = 128 * m if m > 0 else 0
                    kt = kt_tiles[(p, kb // 8)]
                    kcols = slice((kb % 8) * 128, (kb % 8) * 128 + 128)
                    qt = qt_tiles[(p, j // 2)]
                    qcols = slice((j % 2) * 512 + c0, (j % 2) * 512 + 512)
                    sc = psc.tile([128, 2, 512], F32, tag="sc")
                    nc.tensor.matmul(
                        sc[:, 0, c0:512],
                        kt[0:64, kcols],
                        qt[0:64, qcols],
                        start=True, stop=True, tile_position=(0, 0),
                    )
                    nc.tensor.matmul(
                        sc[:, 1, c0:512],
                        kt[64:128, kcols],
                        qt[64:128, qcols],
                        start=True, stop=True, tile_position=(64, 0),
                    )
                    if m >= 0:
                        nc.vector.tensor_add(
                            sc[:, :, c0:c0 + 128], sc[:, :, c0:c0 + 128], mask_sb)
                    et = pe_.tile([128, 2, 512], BF16, tag="e")
                    nc.scalar.activation(
                        et[:, :, c0:512], sc[:, :, c0:512], AF.Exp, scale=0.125)
                    hA, hB = 2 * p, 2 * p + 1
                    nc.tensor.matmul(
                        pvA[0:65, c0:512], v_sb[kb][:, hA, :], et[:, 0, c0:512],
                        start=(kb == 0), stop=(kb == nkb - 1),
                    )
                    nc.tensor.matmul(
                        pvB[0:65, c0:512], v_sb[kb][:, hB, :], et[:, 1, c0:512],
                        start=(kb == 0), stop=(kb == nkb - 1),
                    )
                recA = prec.tile([1, 512], F32, tag="rec")
                recB = prec.tile([1, 512], F32, tag="rec")
                nc.vector.reciprocal(recA, pvA[64:65, :])
                nc.vector.reciprocal(recB, pvB[64:65, :])
                rec_d = pdram.tile([2, 512], F32, tag="recd")
                nc.sync.dma_start(rec_d[0:1, :], recA)
                nc.sync.dma_start(rec_d[1:2, :], recB)
                bcs = ptmp.tile([128, 512], F32, tag="tmp")
                nc.sync.dma_start(bcs[0:64, :], rec_d[0:1, :].to_broadcast([64, 512]))
                nc.sync.dma_start(bcs[64:128, :], rec_d[1:2, :].to_broadcast([64, 512]))
                at = pat.tile([128, 512], F32R, tag="at")
                nc.vector.tensor_mul(at[0:64], pvA[0:64], bcs[0:64])
                nc.vector.tensor_mul(at[64:128], pvB[0:64], bcs[64:128])
                at_tiles[(p, j)] = at

            for et_i in range(NE):
                pso = psc.tile([128, 512], F32, tag="sc")
                for p in range(2):
                    nc.tensor.matmul(
                        pso,
                        wo_sb[p][:, et_i * 128:(et_i + 1) * 128],
                        at_tiles[(p, j)],
                        start=(p == 0), stop=(p == 1),
                    )
                stg = ptmp.tile([128, 1024], F32, tag="tmp")
                nc.vector.tensor_copy(stg[:, 0:512], pso)
                nc.sync.dma_start(
                    d["out_r"][et_i][:, j * 512:(j + 1) * 512], stg[:, 0:512])


def make_host_inputs(x, Wq, bq, Wk, bk, Wv, bv, Wo, bo):
    """Shard + pre-transpose inputs per core. Returns (in_maps, bo)."""
    x = np.asarray(x, np.float32)
    Wq, Wk, Wv, Wo = (np.asarray(w, np.float32) for w in (Wq, Wk, Wv, Wo))
    bq, bk, bv, bo = (np.asarray(b_, np.float32) for b_ in (bq, bk, bv, bo))

    # RoPE tables
    half = D // 2
    inv_freq = 1.0 / (ROPE_BASE ** (np.arange(half, dtype=np.float64) / half))
    pos = np.arange(S, dtype=np.float64)
    sinus = pos[:, None] * inv_freq[None, :]           # [S, 32]
    sin_full = np.repeat(np.sin(sinus), 2, axis=1)     # [S, 64] interleave-dup
    cos_full = np.repeat(np.cos(sinus), 2, axis=1)
    sgn = np.where(np.arange(D) % 2 == 0, -1.0, 1.0)
    cos2 = np.tile(cos_full.T, (2, 1)).astype(np.float32)
    sin2 = np.tile((sin_full * sgn[None, :]).T, (2, 1)).astype(np.float32)

    tri = np.where(np.arange(128)[:, None] <= np.arange(128)[None, :],
                   0.0, MASK_VAL).astype(np.float32)
    mask2 = np.stack([tri, tri], axis=1)               # [128, 2, 128]
    xT = [np.ascontiguousarray(x[b_].T) for b_ in range(B)]
    in_maps = []
    for c in range(8):
        b_, hg = c // 4, c % 4
        rows = slice(DH * hg, DH * hg + DH)
        in_maps.append({
            "xT": xT[b_],
            "wqT": np.ascontiguousarray(Wq[rows].T),
            "wkT": np.ascontiguousarray(Wk[rows].T),
            "wvT": np.ascontiguousarray(Wv[rows].T),
            "woST": np.ascontiguousarray(Wo[:, rows].T),
            "bq2": np.ascontiguousarray(bq[rows].reshape(2, 128).T),
            "bk2": np.ascontiguousarray(bk[rows].reshape(2, 128).T),
            "bvbc": np.tile(bv[rows][None, :], (128, 1)).astype(np.float32),
            "cos2": cos2,
            "sin2": sin2,
            "mask2": mask2,
        })
    return in_maps, bo


_NC_CACHE = {}


def get_nc():
    if "nc" not in _NC_CACHE:
        _NC_CACHE["nc"] = build_nc()
    return _NC_CACHE["nc"]


def kernel(**inputs):
    in_maps, bo = make_host_inputs(**inputs)
    nc = get_nc()
    res = run_bass_kernel_spmd(nc, in_maps, core_ids=list(range(8)))
    out = np.zeros((B, S, E), np.float32)
    for c in range(8):
        out[c // 4] += res.results[c]["out"].T
    out += bo[None, None, :]
    return out


# revision 19
# speedup vs baseline: 1.0921x; 1.0921x over previous
"""Causal RoPE self-attention, distributed over 8 TRN2 NeuronCores.

Sharding: batch (2) x head-groups (4 heads each) -> 8 cores.
Each core computes, for its (batch b, head-group hg):
    q/k/v projections for its 4 heads (tensor-parallel column split),
    RoPE, causal attention, and the row-parallel slice of the output
    projection, producing a partial output partialT = WoS^T @ attnT
    of shape [E, S].  The host sums the 4 partials per batch and adds bo.

On-device layout notes:
  - activations live transposed: qT/kT are [head-dim, seq] so the
    score matmul sT[k, q] = K Q^T contracts over d on partitions, and
    softmax denominators come from an extra all-ones column in V.
  - matmuls run with float32r operand views (full fp32 storage,
    1 cycle/row TensorEngine rate); exp'd scores and V are bf16.
"""

import numpy as np

import concourse.bass as bass
import concourse.tile as tile
from concourse import bacc, mybir
from concourse.bass_utils import run_bass_kernel_spmd

F32 = mybir.dt.float32
F32R = mybir.dt.float32r
BF16 = mybir.dt.bfloat16
AF = mybir.ActivationFunctionType

B, S, E = 2, 2048, 1024
H, D = 16, 64
HPG = 4                # heads per core
DH = HPG * D           # 256 head-dims per core
NE = E // 128          # 8 e-chunks
NST = S // 128         # 16 s-tiles / key blocks
NSL = S // 512         # 4 q-slices
ROPE_BASE = 10000.0
MASK_VAL = -30000.0

_SWAP_MASK = [i ^ 1 for i in range(32)]


def build_nc():
    """Build + compile the per-core Bass graph (same graph on all 8 cores)."""
    nc = bacc.Bacc("TRN2", target_bir_lowering=False, debug=False, num_devices=8)

    def din(name, shape, dt=F32):
        return nc.dram_tensor(name, shape, dt, kind="ExternalInput").ap()

    xT = din("xT", [E, S], F32R)
    wqT = din("wqT", [E, DH], F32R)
    wkT = din("wkT", [E, DH], F32R)
    wvT = din("wvT", [E, DH], F32R)
    woST = din("woST", [DH, E], F32R)
    bq2 = din("bq2", [128, 2])
    bk2 = din("bk2", [128, 2])
    bvbc = din("bvbc", [128, DH])
    cos2 = din("cos2", [128, S])            # cosT duplicated on both halves
    sin2 = din("sin2", [128, S])            # signed sinT duplicated on both halves
    mask2 = din("mask2", [128, 2, 128])     # causal tri mask, duplicated x2
    out = nc.dram_tensor("out", [E, S], F32, kind="ExternalOutput").ap()

    xT_r = xT.rearrange("(n p) s -> n p s", p=128)
    wq_r = wqT.rearrange("(n p) d -> n p d", p=128)
    wk_r = wkT.rearrange("(n p) d -> n p d", p=128)
    wv_r = wvT.rearrange("(n p) d -> n p d", p=128)
    wo_r = woST.rearrange("(n p) e -> n p e", p=128)
    out_r = out.rearrange("(n p) s -> n p s", p=128)

    with tile.TileContext(nc) as tc, nc.allow_low_precision(
            reason="float32r matmul operands (fp32 storage, TF32-rate)"):
        _emit(tc, nc, dict(
            xT_r=xT_r, wq_r=wq_r, wk_r=wk_r, wv_r=wv_r, wo_r=wo_r, out_r=out_r,
            bq2=bq2, bk2=bk2, bvbc=bvbc, cos2=cos2, sin2=sin2, mask2=mask2,
        ))
    nc.compile()
    return nc


def _emit(tc, nc, d):
    from contextlib import ExitStack
    ctx = ExitStack()
    with ctx:
        consts = ctx.enter_context(tc.tile_pool(name="consts", bufs=1))
        px = ctx.enter_context(tc.tile_pool(name="px", bufs=8))
        pwq = ctx.enter_context(tc.tile_pool(name="pwq", bufs=8))
        pwk = ctx.enter_context(tc.tile_pool(name="pwk", bufs=8))
        pwv = ctx.enter_context(tc.tile_pool(name="pwv", bufs=8))
        pwo = ctx.enter_context(tc.tile_pool(name="pwo", bufs=2))
        pqt = ctx.enter_context(tc.tile_pool(name="pqt", bufs=4))
        pkt = ctx.enter_context(tc.tile_pool(name="pkt", bufs=4))
        pv = ctx.enter_context(tc.tile_pool(name="pv", bufs=16))
        pat = ctx.enter_context(tc.tile_pool(name="pat", bufs=4))
        ptmp = ctx.enter_context(tc.tile_pool(name="ptmp", bufs=4))
        pe_ = ctx.enter_context(tc.tile_pool(name="pe", bufs=3))
        prec = ctx.enter_context(tc.tile_pool(name="prec", bufs=4))
        pdram = ctx.enter_context(tc.tile_pool(name="pdram", bufs=4, space="DRAM"))
        psc = ctx.enter_context(tc.tile_pool(name="psc", bufs=2, space="PSUM"))
        ppv = ctx.enter_context(tc.tile_pool(name="ppv", bufs=4, space="PSUM"))

        # ---- constants ----
        cos2_sb = consts.tile([128, S], F32)
        nc.sync.dma_start(cos2_sb, d["cos2"])
        sin2_sb = consts.tile([128, S], F32)
        nc.sync.dma_start(sin2_sb, d["sin2"])
        mask_sb = consts.tile([128, 2, 128], F32)
        nc.sync.dma_start(mask_sb, d["mask2"])
        bq2_sb = consts.tile([128, 2], F32)
        nc.sync.dma_start(bq2_sb, d["bq2"])
        bk2_sb = consts.tile([128, 2], F32)
        nc.sync.dma_start(bk2_sb, d["bk2"])
        bvbc_sb = consts.tile([128, DH], F32)
        nc.sync.dma_start(bvbc_sb, d["bvbc"])

        # ---- weights ----
        wq_sb, wk_sb, wv_sb = [], [], []
        for e in range(NE):
            t = pwq.tile([128, DH], F32R, tag="wq")
            nc.sync.dma_start(t, d["wq_r"][e])
            wq_sb.append(t)
            t = pwk.tile([128, DH], F32R, tag="wk")
            nc.sync.dma_start(t, d["wk_r"][e])
            wk_sb.append(t)
            t = pwv.tile([128, DH], F32R, tag="wv")
            nc.sync.dma_start(t, d["wv_r"][e])
            wv_sb.append(t)
        wo_sb = []
        for p in range(2):
            t = pwo.tile([128, E], F32R, tag="wo")
            nc.sync.dma_start(t, d["wo_r"][p])
            wo_sb.append(t)

        # ---- x^T ----
        xt_sb = []
        for e in range(NE):
            t = px.tile([128, S], F32R, tag="xt")
            nc.sync.dma_start(t, d["xT_r"][e])
            xt_sb.append(t)

        # ---- q/k projections + RoPE ----
        def qk_proj(w_sb, bias_sb, dst_pool, dst_tag):
            tiles = {}
            for p in range(2):
                for sp in range(2):   # slice-pairs of 1024 cols
                    ps = psc.tile([128, 2, 512], F32, tag="sc")
                    for half in range(2):
                        scol = (sp * 2 + half) * 512
                        for e in range(NE):
                            nc.tensor.matmul(
                                ps[:, half, :],
                                w_sb[e][:, p * 128:(p + 1) * 128],
                                xt_sb[e][:, scol:scol + 512],
                                start=(e == 0), stop=(e == NE - 1),
                            )
                    cols = slice(sp * 1024, (sp + 1) * 1024)
                    tq = ptmp.tile([128, 1024], F32, tag="tmp")
                    nc.scalar.activation(
                        tq.rearrange("p (a b) -> p a b", b=512), ps,
                        AF.Identity, bias=bias_sb[:, p:p + 1])
                    tsh = ptmp.tile([128, 1024], F32, tag="tmp")
                    nc.vector.stream_shuffle(tsh, tq, _SWAP_MASK)
                    nc.vector.tensor_mul(tsh, tsh, sin2_sb[:, cols])
                    nc.vector.tensor_mul(tq, tq, cos2_sb[:, cols])
                    qt = dst_pool.tile([128, 1024], F32R, tag=dst_tag)
                    nc.vector.tensor_add(qt, tq, tsh)
                    tiles[(p, sp)] = qt
            return tiles

        qt_tiles = qk_proj(wq_sb, bq2_sb, pqt, "qt")
        kt_tiles = qk_proj(wk_sb, bk2_sb, pkt, "kt")

        # ---- v projection (natural [s, d] layout, bf16, ones col per head) ----
        v_sb = []
        for st in range(NST):
            psv = psc.tile([128, DH], F32, tag="sc")
            for e in range(NE):
                nc.tensor.matmul(
                    psv,
                    xt_sb[e][:, st * 128:(st + 1) * 128],
                    wv_sb[e],
                    start=(e == 0), stop=(e == NE - 1),
                )
            vt = pv.tile([128, HPG, 65], BF16, tag="v")
            nc.vector.memset(vt[:, :, 64:65], 1.0)
            nc.vector.tensor_add(
                vt[:, :, 0:64],
                psv.rearrange("p (h dd) -> p h dd", dd=64),
                bvbc_sb.rearrange("p (h dd) -> p h dd", dd=64),
            )
            v_sb.append(vt)

        # ---- attention + output projection, per q-slice ----
        at_tiles = {}
        for j in range(NSL):
            for p in range(2):
                pvA = ppv.tile([128, 512], F32, tag="ppv")
                pvB = ppv.tile([128, 512], F32, tag="ppv")
                nkb = 4 * j + 4
                for kb in range(nkb):
                    m = kb - 4 * j
                    c0 = 128 * m if m > 0 else 0
                    kt = kt_tiles[(p, kb // 8)]
                    kcols = slice((kb % 8) * 128, (kb % 8) * 128 + 128)
                    qt = qt_tiles[(p, j // 2)]
                    qcols = slice((j % 2) * 512 + c0, (j % 2) * 512 + 512)
                    sc = psc.tile([128, 2, 512], F32, tag="sc")
                    nc.tensor.matmul(
                        sc[:, 0, c0:512],
                        kt[0:64, kcols],
                        qt[0:64, qcols],
                        start=True, stop=True, tile_position=(0, 0),
                    )
                    nc.tensor.matmul(
                        sc[:, 1, c0:512],
                        kt[64:128, kcols],
                        qt[64:128, qcols],
                        start=True, stop=True, tile_position=(64, 0),
                    )
                    if m >= 0:
                        nc.vector.tensor_add(
                            sc[:, :, c0:c0 + 128], sc[:, :, c0:c0 + 128], mask_sb)
                    et = pe_.tile([128, 2, 512], BF16, tag="e")
                    nc.scalar.activation(
                        et[:, :, c0:512], sc[:, :, c0:512], AF.Exp, scale=0.125)
                    hA, hB = 2 * p, 2 * p + 1
                    nc.tensor.matmul(
                        pvA[0:65, c0:512], v_sb[kb][:, hA, :], et[:, 0, c0:512],
                        start=(kb == 0), stop=(kb == nkb - 1),
                    )
                    nc.tensor.matmul(
                        pvB[0:65, c0:512], v_sb[kb][:, hB, :], et[:, 1, c0:512],
                        start=(kb == 0), stop=(kb == nkb - 1),
                    )
                rec = prec.tile([1, 1024], F32, tag="rec")
                nc.vector.reciprocal(rec[:, 0:512], pvA[64:65, :])
                nc.vector.reciprocal(rec[:, 512:1024], pvB[64:65, :])
                rec_d = pdram.tile([1, 1024], F32, tag="recd")
                nc.sync.dma_start(rec_d, rec)
                bcs = ptmp.tile([128, 512], F32, tag="tmp")
                nc.sync.dma_start(bcs[0:64, :], rec_d[:, 0:512].to_broadcast([64, 512]))
                nc.sync.dma_start(bcs[64:128, :], rec_d[:, 512:1024].to_broadcast([64, 512]))
                at = pat.tile([128, 512], F32R, tag="at")
                nc.vector.tensor_mul(at[0:64], pvA[0:64], bcs[0:64])
                nc.vector.tensor_mul(at[64:128], pvB[0:64], bcs[64:128])
                at_tiles[(p, j)] = at

            for et_i in range(NE):
                pso = psc.tile([128, 512], F32, tag="sc")
                for p in range(2):
                    nc.tensor.matmul(
                        pso,
                        wo_sb[p][:, et_i * 128:(et_i + 1) * 128],
                        at_tiles[(p, j)],
                        start=(p == 0), stop=(p == 1),
                    )
                stg = ptmp.tile([128, 1024], F32, tag="tmp")
                nc.vector.tensor_copy(stg[:, 0:512], pso)
                nc.sync.dma_start(
                    d["out_r"][et_i][:, j * 512:(j + 1) * 512], stg[:, 0:512])


def make_host_inputs(x, Wq, bq, Wk, bk, Wv, bv, Wo, bo):
    """Shard + pre-transpose inputs per core. Returns (in_maps, bo)."""
    x = np.asarray(x, np.float32)
    Wq, Wk, Wv, Wo = (np.asarray(w, np.float32) for w in (Wq, Wk, Wv, Wo))
    bq, bk, bv, bo = (np.asarray(b_, np.float32) for b_ in (bq, bk, bv, bo))

    # RoPE tables
    half = D // 2
    inv_freq = 1.0 / (ROPE_BASE ** (np.arange(half, dtype=np.float64) / half))
    pos = np.arange(S, dtype=np.float64)
    sinus = pos[:, None] * inv_freq[None, :]           # [S, 32]
    sin_full = np.repeat(np.sin(sinus), 2, axis=1)     # [S, 64] interleave-dup
    cos_full = np.repeat(np.cos(sinus), 2, axis=1)
    sgn = np.where(np.arange(D) % 2 == 0, -1.0, 1.0)
    cos2 = np.tile(cos_full.T, (2, 1)).astype(np.float32)
    sin2 = np.tile((sin_full * sgn[None, :]).T, (2, 1)).astype(np.float32)

    tri = np.where(np.arange(128)[:, None] <= np.arange(128)[None, :],
                   0.0, MASK_VAL).astype(np.float32)
    mask2 = np.stack([tri, tri], axis=1)               # [128, 2, 128]
    xT = [np.ascontiguousarray(x[b_].T) for b_ in range(B)]
    in_maps = []
    for c in range(8):
        b_, hg = c // 4, c % 4
        rows = slice(DH * hg, DH * hg + DH)
        in_maps.append({
            "xT": xT[b_],
            "wqT": np.ascontiguousarray(Wq[rows].T),
            "wkT": np.ascontiguousarray(Wk[rows].T),
            "wvT": np.ascontiguousarray(Wv[rows].T),
            "woST": np.ascontiguousarray(Wo[:, rows].T),
            "bq2": np.ascontiguousarray(bq[rows].reshape(2, 128).T),
            "bk2": np.ascontiguousarray(bk[rows].reshape(2, 128).T),
            "bvbc": np.tile(bv[rows][None, :], (128, 1)).astype(np.float32),
            "cos2": cos2,
            "sin2": sin2,
            "mask2": mask2,
        })
    return in_maps, bo


_NC_CACHE = {}


def get_nc():
    if "nc" not in _NC_CACHE:
        _NC_CACHE["nc"] = build_nc()
    return _NC_CACHE["nc"]


def kernel(**inputs):
    in_maps, bo = make_host_inputs(**inputs)
    nc = get_nc()
    res = run_bass_kernel_spmd(nc, in_maps, core_ids=list(range(8)))
    out = np.zeros((B, S, E), np.float32)
    for c in range(8):
        out[c // 4] += res.results[c]["out"].T
    out += bo[None, None, :]
    return out


# revision 21
# speedup vs baseline: 1.1580x; 1.0604x over previous
"""Causal RoPE self-attention, distributed over 8 TRN2 NeuronCores.

Sharding: batch (2) x head-groups (4 heads each) -> 8 cores.
Each core computes, for its (batch b, head-group hg):
    q/k/v projections for its 4 heads (tensor-parallel column split),
    RoPE, causal attention, and the row-parallel slice of the output
    projection, producing a partial output partialT = WoS^T @ attnT
    of shape [E, S].  The host sums the 4 partials per batch and adds bo.

On-device layout notes:
  - activations live transposed: qT/kT are [head-dim, seq] so the
    score matmul sT[k, q] = K Q^T contracts over d on partitions, and
    softmax denominators come from an extra all-ones column in V.
  - matmuls run with float32r operand views (full fp32 storage,
    1 cycle/row TensorEngine rate); exp'd scores and V are bf16.
"""

import numpy as np

import concourse.bass as bass
import concourse.tile as tile
from concourse import bacc, mybir
from concourse.bass_utils import run_bass_kernel_spmd

F32 = mybir.dt.float32
F32R = mybir.dt.float32r
BF16 = mybir.dt.bfloat16
AF = mybir.ActivationFunctionType

B, S, E = 2, 2048, 1024
H, D = 16, 64
HPG = 4                # heads per core
DH = HPG * D           # 256 head-dims per core
NE = E // 128          # 8 e-chunks
NST = S // 128         # 16 s-tiles / key blocks
NSL = S // 512         # 4 q-slices
ROPE_BASE = 10000.0
MASK_VAL = -30000.0

_SWAP_MASK = [i ^ 1 for i in range(32)]


def build_nc():
    """Build + compile the per-core Bass graph (same graph on all 8 cores)."""
    nc = bacc.Bacc("TRN2", target_bir_lowering=False, debug=False, num_devices=8)

    def din(name, shape, dt=F32):
        return nc.dram_tensor(name, shape, dt, kind="ExternalInput").ap()

    xT = din("xT", [E, S], F32R)
    wqT = din("wqT", [E, DH], F32R)
    wkT = din("wkT", [E, DH], F32R)
    wvT = din("wvT", [E, DH], F32R)
    woST = din("woST", [DH, E], F32R)
    bq2 = din("bq2", [128, 2])
    bk2 = din("bk2", [128, 2])
    bvbc = din("bvbc", [128, DH])
    cos2 = din("cos2", [128, S])            # cosT duplicated on both halves
    sin2 = din("sin2", [128, S])            # signed sinT duplicated on both halves
    mask2 = din("mask2", [128, 2, 128])     # causal tri mask, duplicated x2
    out = nc.dram_tensor("out", [E, S], F32, kind="ExternalOutput").ap()

    xT_r = xT.rearrange("(n p) s -> n p s", p=128)
    wq_r = wqT.rearrange("(n p) d -> n p d", p=128)
    wk_r = wkT.rearrange("(n p) d -> n p d", p=128)
    wv_r = wvT.rearrange("(n p) d -> n p d", p=128)
    wo_r = woST.rearrange("(n p) e -> n p e", p=128)
    out_r = out.rearrange("(n p) s -> n p s", p=128)

    with tile.TileContext(nc) as tc, nc.allow_low_precision(
            reason="float32r matmul operands (fp32 storage, TF32-rate)"):
        _emit(tc, nc, dict(
            xT_r=xT_r, wq_r=wq_r, wk_r=wk_r, wv_r=wv_r, wo_r=wo_r, out_r=out_r,
            bq2=bq2, bk2=bk2, bvbc=bvbc, cos2=cos2, sin2=sin2, mask2=mask2,
        ))
    nc.compile()
    return nc


def _emit(tc, nc, d):
    from contextlib import ExitStack
    ctx = ExitStack()
    with ctx:
        consts = ctx.enter_context(tc.tile_pool(name="consts", bufs=1))
        px = ctx.enter_context(tc.tile_pool(name="px", bufs=8))
        pwq = ctx.enter_context(tc.tile_pool(name="pwq", bufs=8))
        pwk = ctx.enter_context(tc.tile_pool(name="pwk", bufs=8))
        pwv = ctx.enter_context(tc.tile_pool(name="pwv", bufs=8))
        pwo = ctx.enter_context(tc.tile_pool(name="pwo", bufs=2))
        pqt = ctx.enter_context(tc.tile_pool(name="pqt", bufs=4))
        pkt = ctx.enter_context(tc.tile_pool(name="pkt", bufs=4))
        pv = ctx.enter_context(tc.tile_pool(name="pv", bufs=16))
        pat = ctx.enter_context(tc.tile_pool(name="pat", bufs=4))
        ptmp = ctx.enter_context(tc.tile_pool(name="ptmp", bufs=4))
        pe_ = ctx.enter_context(tc.tile_pool(name="pe", bufs=3))
        prec = ctx.enter_context(tc.tile_pool(name="prec", bufs=2))
        pdram = ctx.enter_context(tc.tile_pool(name="pdram", bufs=4, space="DRAM"))
        psc = ctx.enter_context(tc.tile_pool(name="psc", bufs=2, space="PSUM"))
        ppv = ctx.enter_context(tc.tile_pool(name="ppv", bufs=4, space="PSUM"))

        # ---- constants ----
        cos2_sb = consts.tile([128, S], F32)
        nc.sync.dma_start(cos2_sb, d["cos2"])
        sin2_sb = consts.tile([128, S], F32)
        nc.sync.dma_start(sin2_sb, d["sin2"])
        mask_sb = consts.tile([128, 2, 128], F32)
        nc.sync.dma_start(mask_sb, d["mask2"])
        bq2_sb = consts.tile([128, 2], F32)
        nc.sync.dma_start(bq2_sb, d["bq2"])
        bk2_sb = consts.tile([128, 2], F32)
        nc.sync.dma_start(bk2_sb, d["bk2"])
        bvbc_sb = consts.tile([128, DH], F32)
        nc.sync.dma_start(bvbc_sb, d["bvbc"])

        # ---- weights ----
        wq_sb, wk_sb, wv_sb = [], [], []
        for e in range(NE):
            t = pwq.tile([128, DH], F32R, tag="wq")
            nc.sync.dma_start(t, d["wq_r"][e])
            wq_sb.append(t)
            t = pwk.tile([128, DH], F32R, tag="wk")
            nc.sync.dma_start(t, d["wk_r"][e])
            wk_sb.append(t)
            t = pwv.tile([128, DH], F32R, tag="wv")
            nc.sync.dma_start(t, d["wv_r"][e])
            wv_sb.append(t)
        wo_sb = []
        for p in range(2):
            t = pwo.tile([128, E], F32R, tag="wo")
            nc.sync.dma_start(t, d["wo_r"][p])
            wo_sb.append(t)

        # ---- x^T ----
        xt_sb = []
        for e in range(NE):
            t = px.tile([128, S], F32R, tag="xt")
            nc.sync.dma_start(t, d["xT_r"][e])
            xt_sb.append(t)

        # ---- q/k projections + RoPE ----
        def qk_proj(w_sb, bias_sb, dst_pool, dst_tag):
            tiles = {}
            for p in range(2):
                for sp in range(2):   # slice-pairs of 1024 cols
                    ps = psc.tile([128, 2, 512], F32, tag="sc")
                    for half in range(2):
                        scol = (sp * 2 + half) * 512
                        for e in range(NE):
                            nc.tensor.matmul(
                                ps[:, half, :],
                                w_sb[e][:, p * 128:(p + 1) * 128],
                                xt_sb[e][:, scol:scol + 512],
                                start=(e == 0), stop=(e == NE - 1),
                            )
                    cols = slice(sp * 1024, (sp + 1) * 1024)
                    tq = ptmp.tile([128, 1024], F32, tag="tmp")
                    nc.scalar.activation(
                        tq.rearrange("p (a b) -> p a b", b=512), ps,
                        AF.Identity, bias=bias_sb[:, p:p + 1])
                    tsh = ptmp.tile([128, 1024], F32, tag="tmp")
                    nc.vector.stream_shuffle(tsh, tq, _SWAP_MASK)
                    nc.vector.tensor_mul(tsh, tsh, sin2_sb[:, cols])
                    nc.vector.tensor_mul(tq, tq, cos2_sb[:, cols])
                    qt = dst_pool.tile([128, 1024], F32R, tag=dst_tag)
                    nc.vector.tensor_add(qt, tq, tsh)
                    tiles[(p, sp)] = qt
            return tiles

        qt_tiles = qk_proj(wq_sb, bq2_sb, pqt, "qt")
        kt_tiles = qk_proj(wk_sb, bk2_sb, pkt, "kt")

        # ---- v projection (natural [s, d] layout, bf16, ones col per head) ----
        v_sb = []
        for st in range(NST):
            psv = psc.tile([128, DH], F32, tag="sc")
            for e in range(NE):
                nc.tensor.matmul(
                    psv,
                    xt_sb[e][:, st * 128:(st + 1) * 128],
                    wv_sb[e],
                    start=(e == 0), stop=(e == NE - 1),
                )
            vt = pv.tile([128, HPG, 65], BF16, tag="v")
            nc.vector.memset(vt[:, :, 64:65], 1.0)
            nc.vector.tensor_add(
                vt[:, :, 0:64],
                psv.rearrange("p (h dd) -> p h dd", dd=64),
                bvbc_sb.rearrange("p (h dd) -> p h dd", dd=64),
            )
            v_sb.append(vt)

        # ---- attention + output projection, per q-slice ----
        at_tiles = {}
        for j in range(NSL):
            for p in range(2):
                pvA = ppv.tile([128, 512], F32, tag="ppv")
                pvB = ppv.tile([128, 512], F32, tag="ppv")
                nkb = 4 * j + 4
                for kb in range(nkb):
                    m = kb - 4 * j
                    c0 = 128 * m if m > 0 else 0
                    kt = kt_tiles[(p, kb // 8)]
                    kcols = slice((kb % 8) * 128, (kb % 8) * 128 + 128)
                    qt = qt_tiles[(p, j // 2)]
                    qcols = slice((j % 2) * 512 + c0, (j % 2) * 512 + 512)
                    sc = psc.tile([128, 2, 512], F32, tag="sc")
                    nc.tensor.matmul(
                        sc[:, 0, c0:512],
                        kt[0:64, kcols],
                        qt[0:64, qcols],
                        start=True, stop=True, tile_position=(0, 0),
                    )
                    nc.tensor.matmul(
                        sc[:, 1, c0:512],
                        kt[64:128, kcols],
                        qt[64:128, qcols],
                        start=True, stop=True, tile_position=(64, 0),
                    )
                    if m >= 0:
                        nc.vector.tensor_add(
                            sc[:, :, c0:c0 + 128], sc[:, :, c0:c0 + 128], mask_sb)
                    et = pe_.tile([128, 2, 512], BF16, tag="e")
                    nc.scalar.activation(
                        et[:, :, c0:512], sc[:, :, c0:512], AF.Exp, scale=0.125)
                    hA, hB = 2 * p, 2 * p + 1
                    nc.tensor.matmul(
                        pvA[0:65, c0:512], v_sb[kb][:, hA, :], et[:, 0, c0:512],
                        start=(kb == 0), stop=(kb == nkb - 1),
                    )
                    nc.tensor.matmul(
                        pvB[0:65, c0:512], v_sb[kb][:, hB, :], et[:, 1, c0:512],
                        start=(kb == 0), stop=(kb == nkb - 1),
                    )
                den = prec.tile([1, 1024], F32, tag="den")
                nc.vector.tensor_copy(den[:, 0:512], pvA[64:65, :])
                nc.vector.tensor_copy(den[:, 512:1024], pvB[64:65, :])
                rec = prec.tile([1, 1024], F32, tag="rec")
                nc.vector.reciprocal_approx_fast(rec, den)
                rec_d = pdram.tile([1, 1024], F32, tag="recd")
                nc.sync.dma_start(rec_d, rec)
                bcs = ptmp.tile([128, 512], F32, tag="tmp")
                nc.sync.dma_start(bcs[0:64, :], rec_d[:, 0:512].to_broadcast([64, 512]))
                nc.sync.dma_start(bcs[64:128, :], rec_d[:, 512:1024].to_broadcast([64, 512]))
                at = pat.tile([128, 512], F32R, tag="at")
                nc.vector.tensor_mul(at[0:64], pvA[0:64], bcs[0:64])
                nc.vector.tensor_mul(at[64:128], pvB[0:64], bcs[64:128])
                at_tiles[(p, j)] = at

            for et_i in range(NE):
                pso = psc.tile([128, 512], F32, tag="sc")
                for p in range(2):
                    nc.tensor.matmul(
                        pso,
                        wo_sb[p][:, et_i * 128:(et_i + 1) * 128],
                        at_tiles[(p, j)],
                        start=(p == 0), stop=(p == 1),
                    )
                stg = ptmp.tile([128, 1024], F32, tag="tmp")
                nc.vector.tensor_copy(stg[:, 0:512], pso)
                nc.sync.dma_start(
                    d["out_r"][et_i][:, j * 512:(j + 1) * 512], stg[:, 0:512])


def make_host_inputs(x, Wq, bq, Wk, bk, Wv, bv, Wo, bo):
    """Shard + pre-transpose inputs per core. Returns (in_maps, bo)."""
    x = np.asarray(x, np.float32)
    Wq, Wk, Wv, Wo = (np.asarray(w, np.float32) for w in (Wq, Wk, Wv, Wo))
    bq, bk, bv, bo = (np.asarray(b_, np.float32) for b_ in (bq, bk, bv, bo))

    # RoPE tables
    half = D // 2
    inv_freq = 1.0 / (ROPE_BASE ** (np.arange(half, dtype=np.float64) / half))
    pos = np.arange(S, dtype=np.float64)
    sinus = pos[:, None] * inv_freq[None, :]           # [S, 32]
    sin_full = np.repeat(np.sin(sinus), 2, axis=1)     # [S, 64] interleave-dup
    cos_full = np.repeat(np.cos(sinus), 2, axis=1)
    sgn = np.where(np.arange(D) % 2 == 0, -1.0, 1.0)
    cos2 = np.tile(cos_full.T, (2, 1)).astype(np.float32)
    sin2 = np.tile((sin_full * sgn[None, :]).T, (2, 1)).astype(np.float32)

    tri = np.where(np.arange(128)[:, None] <= np.arange(128)[None, :],
                   0.0, MASK_VAL).astype(np.float32)
    mask2 = np.stack([tri, tri], axis=1)               # [128, 2, 128]
    xT = [np.ascontiguousarray(x[b_].T) for b_ in range(B)]
    in_maps = []
    for c in range(8):
        b_, hg = c // 4, c % 4
        rows = slice(DH * hg, DH * hg + DH)
        in_maps.append({
            "xT": xT[b_],
            "wqT": np.ascontiguousarray(Wq[rows].T),
            "wkT": np.ascontiguousarray(Wk[rows].T),
            "wvT": np.ascontiguousarray(Wv[rows].T),
            "woST": np.ascontiguousarray(Wo[:, rows].T),
            "bq2": np.ascontiguousarray(bq[rows].reshape(2, 128).T),
            "bk2": np.ascontiguousarray(bk[rows].reshape(2, 128).T),
            "bvbc": np.tile(bv[rows][None, :], (128, 1)).astype(np.float32),
            "cos2": cos2,
            "sin2": sin2,
            "mask2": mask2,
        })
    return in_maps, bo


_NC_CACHE = {}


def get_nc():
    if "nc" not in _NC_CACHE:
        _NC_CACHE["nc"] = build_nc()
    return _NC_CACHE["nc"]


def kernel(**inputs):
    in_maps, bo = make_host_inputs(**inputs)
    nc = get_nc()
    res = run_bass_kernel_spmd(nc, in_maps, core_ids=list(range(8)))
    out = np.zeros((B, S, E), np.float32)
    for c in range(8):
        out[c // 4] += res.results[c]["out"].T
    out += bo[None, None, :]
    return out


# revision 22
# speedup vs baseline: 1.1775x; 1.0168x over previous
"""Causal RoPE self-attention, distributed over 8 TRN2 NeuronCores.

Sharding: batch (2) x head-groups (4 heads each) -> 8 cores.
Each core computes, for its (batch b, head-group hg):
    q/k/v projections for its 4 heads (tensor-parallel column split),
    RoPE, causal attention, and the row-parallel slice of the output
    projection, producing a partial output partialT = WoS^T @ attnT
    of shape [E, S].  The host sums the 4 partials per batch and adds bo.

On-device layout notes:
  - activations live transposed: qT/kT are [head-dim, seq] so the
    score matmul sT[k, q] = K Q^T contracts over d on partitions, and
    softmax denominators come from an extra all-ones column in V.
  - matmuls run with float32r operand views (full fp32 storage,
    1 cycle/row TensorEngine rate); exp'd scores and V are bf16.
"""

import numpy as np

import concourse.bass as bass
import concourse.tile as tile
from concourse import bacc, mybir
from concourse.bass_utils import run_bass_kernel_spmd

F32 = mybir.dt.float32
F32R = mybir.dt.float32r
BF16 = mybir.dt.bfloat16
AF = mybir.ActivationFunctionType

B, S, E = 2, 2048, 1024
H, D = 16, 64
HPG = 4                # heads per core
DH = HPG * D           # 256 head-dims per core
NE = E // 128          # 8 e-chunks
NST = S // 128         # 16 s-tiles / key blocks
NSL = S // 512         # 4 q-slices
ROPE_BASE = 10000.0
MASK_VAL = -30000.0

_SWAP_MASK = [i ^ 1 for i in range(32)]


def build_nc():
    """Build + compile the per-core Bass graph (same graph on all 8 cores)."""
    nc = bacc.Bacc("TRN2", target_bir_lowering=False, debug=False, num_devices=8)

    def din(name, shape, dt=F32):
        return nc.dram_tensor(name, shape, dt, kind="ExternalInput").ap()

    xT = din("xT", [E, S], F32R)
    wqT = din("wqT", [E, DH], F32R)
    wkT = din("wkT", [E, DH], F32R)
    wvT = din("wvT", [E, DH], F32R)
    woST = din("woST", [DH, E], F32R)
    bq2 = din("bq2", [128, 2])
    bk2 = din("bk2", [128, 2])
    bvbc = din("bvbc", [128, DH])
    cos2 = din("cos2", [128, S])            # cosT duplicated on both halves
    sin2 = din("sin2", [128, S])            # signed sinT duplicated on both halves
    mask2 = din("mask2", [128, 2, 128])     # causal tri mask, duplicated x2
    out = nc.dram_tensor("out", [E, S], F32, kind="ExternalOutput").ap()

    xT_r = xT.rearrange("(n p) s -> n p s", p=128)
    wq_r = wqT.rearrange("(n p) d -> n p d", p=128)
    wk_r = wkT.rearrange("(n p) d -> n p d", p=128)
    wv_r = wvT.rearrange("(n p) d -> n p d", p=128)
    wo_r = woST.rearrange("(n p) e -> n p e", p=128)
    out_r = out.rearrange("(n p) s -> n p s", p=128)

    with tile.TileContext(nc) as tc, nc.allow_low_precision(
            reason="float32r matmul operands (fp32 storage, TF32-rate)"):
        _emit(tc, nc, dict(
            xT_r=xT_r, wq_r=wq_r, wk_r=wk_r, wv_r=wv_r, wo_r=wo_r, out_r=out_r,
            bq2=bq2, bk2=bk2, bvbc=bvbc, cos2=cos2, sin2=sin2, mask2=mask2,
        ))
    nc.compile()
    return nc


def _emit(tc, nc, d):
    from contextlib import ExitStack
    ctx = ExitStack()
    with ctx:
        consts = ctx.enter_context(tc.tile_pool(name="consts", bufs=1))
        px = ctx.enter_context(tc.tile_pool(name="px", bufs=8))
        pwq = ctx.enter_context(tc.tile_pool(name="pwq", bufs=8))
        pwk = ctx.enter_context(tc.tile_pool(name="pwk", bufs=8))
        pwv = ctx.enter_context(tc.tile_pool(name="pwv", bufs=8))
        pwo = ctx.enter_context(tc.tile_pool(name="pwo", bufs=2))
        pqt = ctx.enter_context(tc.tile_pool(name="pqt", bufs=4))
        pkt = ctx.enter_context(tc.tile_pool(name="pkt", bufs=4))
        pv = ctx.enter_context(tc.tile_pool(name="pv", bufs=16))
        pat = ctx.enter_context(tc.tile_pool(name="pat", bufs=4))
        ptmp = ctx.enter_context(tc.tile_pool(name="ptmp", bufs=4))
        pe_ = ctx.enter_context(tc.tile_pool(name="pe", bufs=3))
        prec = ctx.enter_context(tc.tile_pool(name="prec", bufs=2))
        pdram = ctx.enter_context(tc.tile_pool(name="pdram", bufs=4, space="DRAM"))
        psc = ctx.enter_context(tc.tile_pool(name="psc", bufs=2, space="PSUM"))
        ppv = ctx.enter_context(tc.tile_pool(name="ppv", bufs=4, space="PSUM"))

        # ---- constants ----
        cos2_sb = consts.tile([128, S], F32)
        nc.sync.dma_start(cos2_sb, d["cos2"])
        sin2_sb = consts.tile([128, S], F32)
        nc.sync.dma_start(sin2_sb, d["sin2"])
        mask_sb = consts.tile([128, 2, 128], F32)
        nc.sync.dma_start(mask_sb, d["mask2"])
        bq2_sb = consts.tile([128, 2], F32)
        nc.sync.dma_start(bq2_sb, d["bq2"])
        bk2_sb = consts.tile([128, 2], F32)
        nc.sync.dma_start(bk2_sb, d["bk2"])
        bvbc_sb = consts.tile([128, DH], F32)
        nc.sync.dma_start(bvbc_sb, d["bvbc"])

        # ---- weights ----
        wq_sb, wk_sb, wv_sb = [], [], []
        for e in range(NE):
            t = pwq.tile([128, DH], F32R, tag="wq")
            nc.sync.dma_start(t, d["wq_r"][e])
            wq_sb.append(t)
            t = pwk.tile([128, DH], F32R, tag="wk")
            nc.sync.dma_start(t, d["wk_r"][e])
            wk_sb.append(t)
            t = pwv.tile([128, DH], F32R, tag="wv")
            nc.sync.dma_start(t, d["wv_r"][e])
            wv_sb.append(t)
        wo_sb = []
        for p in range(2):
            t = pwo.tile([128, E], F32R, tag="wo")
            nc.sync.dma_start(t, d["wo_r"][p])
            wo_sb.append(t)

        # ---- x^T ----
        xt_sb = []
        for e in range(NE):
            t = px.tile([128, S], F32R, tag="xt")
            nc.sync.dma_start(t, d["xT_r"][e])
            xt_sb.append(t)

        # ---- emission helpers (interleaved so PE never starves while the
        # ---- scalar engine works through the softmax exps) ----
        qt_tiles, kt_tiles, at_tiles = {}, {}, {}
        v_sb = {}

        def emit_qk(w_sb, bias_sb, dst_pool, dst_tag, tiles, p, sp):
            ps = psc.tile([128, 2, 512], F32, tag="sc")
            for half in range(2):
                scol = (sp * 2 + half) * 512
                for e in range(NE):
                    nc.tensor.matmul(
                        ps[:, half, :],
                        w_sb[e][:, p * 128:(p + 1) * 128],
                        xt_sb[e][:, scol:scol + 512],
                        start=(e == 0), stop=(e == NE - 1),
                    )
            cols = slice(sp * 1024, (sp + 1) * 1024)
            tq = ptmp.tile([128, 1024], F32, tag="tmp")
            nc.scalar.activation(
                tq.rearrange("p (a b) -> p a b", b=512), ps,
                AF.Identity, bias=bias_sb[:, p:p + 1])
            tsh = ptmp.tile([128, 1024], F32, tag="tmp")
            nc.vector.stream_shuffle(tsh, tq, _SWAP_MASK)
            nc.vector.tensor_mul(tsh, tsh, sin2_sb[:, cols])
            nc.vector.tensor_mul(tq, tq, cos2_sb[:, cols])
            qt = dst_pool.tile([128, 1024], F32R, tag=dst_tag)
            nc.vector.tensor_add(qt, tq, tsh)
            tiles[(p, sp)] = qt

        def emit_v(st):
            psv = psc.tile([128, DH], F32, tag="sc")
            for e in range(NE):
                nc.tensor.matmul(
                    psv,
                    xt_sb[e][:, st * 128:(st + 1) * 128],
                    wv_sb[e],
                    start=(e == 0), stop=(e == NE - 1),
                )
            vt = pv.tile([128, HPG, 65], BF16, tag="v")
            nc.vector.memset(vt[:, :, 64:65], 1.0)
            nc.vector.tensor_add(
                vt[:, :, 0:64],
                psv.rearrange("p (h dd) -> p h dd", dd=64),
                bvbc_sb.rearrange("p (h dd) -> p h dd", dd=64),
            )
            v_sb[st] = vt

        def emit_attn(p, j):
            pvA = ppv.tile([128, 512], F32, tag="ppv")
            pvB = ppv.tile([128, 512], F32, tag="ppv")
            nkb = 4 * j + 4
            for kb in range(nkb):
                m = kb - 4 * j
                c0 = 128 * m if m > 0 else 0
                kt = kt_tiles[(p, kb // 8)]
                kcols = slice((kb % 8) * 128, (kb % 8) * 128 + 128)
                qt = qt_tiles[(p, j // 2)]
                qcols = slice((j % 2) * 512 + c0, (j % 2) * 512 + 512)
                sc = psc.tile([128, 2, 512], F32, tag="sc")
                nc.tensor.matmul(
                    sc[:, 0, c0:512],
                    kt[0:64, kcols],
                    qt[0:64, qcols],
                    start=True, stop=True, tile_position=(0, 0),
                )
                nc.tensor.matmul(
                    sc[:, 1, c0:512],
                    kt[64:128, kcols],
                    qt[64:128, qcols],
                    start=True, stop=True, tile_position=(64, 0),
                )
                if m >= 0:
                    nc.vector.tensor_add(
                        sc[:, :, c0:c0 + 128], sc[:, :, c0:c0 + 128], mask_sb)
                et = pe_.tile([128, 2, 512], BF16, tag="e")
                nc.scalar.activation(
                    et[:, :, c0:512], sc[:, :, c0:512], AF.Exp, scale=0.125)
                hA, hB = 2 * p, 2 * p + 1
                nc.tensor.matmul(
                    pvA[0:65, c0:512], v_sb[kb][:, hA, :], et[:, 0, c0:512],
                    start=(kb == 0), stop=(kb == nkb - 1),
                )
                nc.tensor.matmul(
                    pvB[0:65, c0:512], v_sb[kb][:, hB, :], et[:, 1, c0:512],
                    start=(kb == 0), stop=(kb == nkb - 1),
                )
            den = prec.tile([1, 1024], F32, tag="den")
            nc.vector.tensor_copy(den[:, 0:512], pvA[64:65, :])
            nc.vector.tensor_copy(den[:, 512:1024], pvB[64:65, :])
            rec = prec.tile([1, 1024], F32, tag="rec")
            nc.vector.reciprocal_approx_fast(rec, den)
            rec_d = pdram.tile([1, 1024], F32, tag="recd")
            nc.sync.dma_start(rec_d, rec)
            bcs = ptmp.tile([128, 512], F32, tag="tmp")
            nc.sync.dma_start(bcs[0:64, :], rec_d[:, 0:512].to_broadcast([64, 512]))
            nc.sync.dma_start(bcs[64:128, :], rec_d[:, 512:1024].to_broadcast([64, 512]))
            at = pat.tile([128, 512], F32R, tag="at")
            nc.vector.tensor_mul(at[0:64], pvA[0:64], bcs[0:64])
            nc.vector.tensor_mul(at[64:128], pvB[0:64], bcs[64:128])
            at_tiles[(p, j)] = at

        def emit_outproj(j):
            for et_i in range(NE):
                pso = ppv.tile([128, 512], F32, tag="ppv")
                for p in range(2):
                    nc.tensor.matmul(
                        pso,
                        wo_sb[p][:, et_i * 128:(et_i + 1) * 128],
                        at_tiles[(p, j)],
                        start=(p == 0), stop=(p == 1),
                    )
                stg = ptmp.tile([128, 1024], F32, tag="tmp")
                nc.vector.tensor_copy(stg[:, 0:512], pso)
                nc.sync.dma_start(
                    d["out_r"][et_i][:, j * 512:(j + 1) * 512], stg[:, 0:512])

        # ---- interleaved schedule ----
        emit_qk(wq_sb, bq2_sb, pqt, "qt", qt_tiles, 0, 0)
        emit_qk(wk_sb, bk2_sb, pkt, "kt", kt_tiles, 0, 0)
        for st in range(0, 4):
            emit_v(st)
        emit_attn(0, 0)
        emit_qk(wq_sb, bq2_sb, pqt, "qt", qt_tiles, 0, 1)
        emit_qk(wk_sb, bk2_sb, pkt, "kt", kt_tiles, 0, 1)
        for st in range(4, 8):
            emit_v(st)
        emit_qk(wq_sb, bq2_sb, pqt, "qt", qt_tiles, 1, 0)
        emit_qk(wk_sb, bk2_sb, pkt, "kt", kt_tiles, 1, 0)
        emit_attn(1, 0)
        emit_outproj(0)
        emit_qk(wq_sb, bq2_sb, pqt, "qt", qt_tiles, 1, 1)
        emit_qk(wk_sb, bk2_sb, pkt, "kt", kt_tiles, 1, 1)
        for st in range(8, 12):
            emit_v(st)
        emit_attn(0, 1)
        emit_attn(1, 1)
        emit_outproj(1)
        for st in range(12, 16):
            emit_v(st)
        emit_attn(0, 2)
        emit_attn(1, 2)
        emit_outproj(2)
        emit_attn(0, 3)
        emit_attn(1, 3)
        emit_outproj(3)


def make_host_inputs(x, Wq, bq, Wk, bk, Wv, bv, Wo, bo):
    """Shard + pre-transpose inputs per core. Returns (in_maps, bo)."""
    x = np.asarray(x, np.float32)
    Wq, Wk, Wv, Wo = (np.asarray(w, np.float32) for w in (Wq, Wk, Wv, Wo))
    bq, bk, bv, bo = (np.asarray(b_, np.float32) for b_ in (bq, bk, bv, bo))

    # RoPE tables
    half = D // 2
    inv_freq = 1.0 / (ROPE_BASE ** (np.arange(half, dtype=np.float64) / half))
    pos = np.arange(S, dtype=np.float64)
    sinus = pos[:, None] * inv_freq[None, :]           # [S, 32]
    sin_full = np.repeat(np.sin(sinus), 2, axis=1)     # [S, 64] interleave-dup
    cos_full = np.repeat(np.cos(sinus), 2, axis=1)
    sgn = np.where(np.arange(D) % 2 == 0, -1.0, 1.0)
    cos2 = np.tile(cos_full.T, (2, 1)).astype(np.float32)
    sin2 = np.tile((sin_full * sgn[None, :]).T, (2, 1)).astype(np.float32)

    tri = np.where(np.arange(128)[:, None] <= np.arange(128)[None, :],
                   0.0, MASK_VAL).astype(np.float32)
    mask2 = np.stack([tri, tri], axis=1)               # [128, 2, 128]
    xT = [np.ascontiguousarray(x[b_].T) for b_ in range(B)]
    in_maps = []
    for c in range(8):
        b_, hg = c // 4, c % 4
        rows = slice(DH * hg, DH * hg + DH)
        in_maps.append({
            "xT": xT[b_],
            "wqT": np.ascontiguousarray(Wq[rows].T),
            "wkT": np.ascontiguousarray(Wk[rows].T),
            "wvT": np.ascontiguousarray(Wv[rows].T),
            "woST": np.ascontiguousarray(Wo[:, rows].T),
            "bq2": np.ascontiguousarray(bq[rows].reshape(2, 128).T),
            "bk2": np.ascontiguousarray(bk[rows].reshape(2, 128).T),
            "bvbc": np.tile(bv[rows][None, :], (128, 1)).astype(np.float32),
            "cos2": cos2,
            "sin2": sin2,
            "mask2": mask2,
        })
    return in_maps, bo


_NC_CACHE = {}


def get_nc():
    if "nc" not in _NC_CACHE:
        _NC_CACHE["nc"] = build_nc()
    return _NC_CACHE["nc"]


def kernel(**inputs):
    in_maps, bo = make_host_inputs(**inputs)
    nc = get_nc()
    res = run_bass_kernel_spmd(nc, in_maps, core_ids=list(range(8)))
    out = np.zeros((B, S, E), np.float32)
    for c in range(8):
        out[c // 4] += res.results[c]["out"].T
    out += bo[None, None, :]
    return out


# revision 23
# speedup vs baseline: 1.2768x; 1.0843x over previous
"""Causal RoPE self-attention, distributed over 8 TRN2 NeuronCores.

Sharding: batch (2) x head-groups (4 heads each) -> 8 cores.
Each core computes, for its (batch b, head-group hg):
    q/k/v projections for its 4 heads (tensor-parallel column split),
    RoPE, causal attention, and the row-parallel slice of the output
    projection, producing a partial output partialT = WoS^T @ attnT
    of shape [E, S].  The host sums the 4 partials per batch and adds bo.

On-device layout notes:
  - activations live transposed: qT/kT are [head-dim, seq] so the
    score matmul sT[k, q] = K Q^T contracts over d on partitions, and
    softmax denominators come from an extra all-ones column in V.
  - matmuls run with float32r operand views (full fp32 storage,
    1 cycle/row TensorEngine rate); exp'd scores and V are bf16.
"""

import ml_dtypes
import numpy as np

import concourse.bass as bass
import concourse.tile as tile
from concourse import bacc, mybir
from concourse.bass_utils import run_bass_kernel_spmd

F32 = mybir.dt.float32
F32R = mybir.dt.float32r
BF16 = mybir.dt.bfloat16
AF = mybir.ActivationFunctionType

B, S, E = 2, 2048, 1024
H, D = 16, 64
HPG = 4                # heads per core
DH = HPG * D           # 256 head-dims per core
NE = E // 128          # 8 e-chunks
NST = S // 128         # 16 s-tiles / key blocks
NSL = S // 512         # 4 q-slices
ROPE_BASE = 10000.0
MASK_VAL = -30000.0

_SWAP_MASK = [i ^ 1 for i in range(32)]


def build_nc():
    """Build + compile the per-core Bass graph (same graph on all 8 cores)."""
    nc = bacc.Bacc("TRN2", target_bir_lowering=False, debug=False, num_devices=8)

    def din(name, shape, dt=F32):
        return nc.dram_tensor(name, shape, dt, kind="ExternalInput").ap()

    xT = din("xT", [E, S], BF16)
    wqT = din("wqT", [E, DH], BF16)
    wkT = din("wkT", [E, DH], BF16)
    wvT = din("wvT", [E, DH], BF16)
    woST = din("woST", [DH, E], BF16)
    bq2 = din("bq2", [128, 2])
    bk2 = din("bk2", [128, 2])
    bvbc = din("bvbc", [128, DH])
    cos2 = din("cos2", [128, S])            # cosT duplicated on both halves
    sin2 = din("sin2", [128, S])            # signed sinT duplicated on both halves
    mask2 = din("mask2", [128, 2, 128])     # causal tri mask, duplicated x2
    out = nc.dram_tensor("out", [E, S], F32, kind="ExternalOutput").ap()

    xT_r = xT.rearrange("(n p) s -> n p s", p=128)
    wq_r = wqT.rearrange("(n p) d -> n p d", p=128)
    wk_r = wkT.rearrange("(n p) d -> n p d", p=128)
    wv_r = wvT.rearrange("(n p) d -> n p d", p=128)
    wo_r = woST.rearrange("(n p) e -> n p e", p=128)
    out_r = out.rearrange("(n p) s -> n p s", p=128)

    with tile.TileContext(nc) as tc, nc.allow_low_precision(
            reason="float32r matmul operands (fp32 storage, TF32-rate)"):
        _emit(tc, nc, dict(
            xT_r=xT_r, wq_r=wq_r, wk_r=wk_r, wv_r=wv_r, wo_r=wo_r, out_r=out_r,
            bq2=bq2, bk2=bk2, bvbc=bvbc, cos2=cos2, sin2=sin2, mask2=mask2,
        ))
    nc.compile()
    return nc


def _emit(tc, nc, d):
    from contextlib import ExitStack
    ctx = ExitStack()
    with ctx:
        consts = ctx.enter_context(tc.tile_pool(name="consts", bufs=1))
        px = ctx.enter_context(tc.tile_pool(name="px", bufs=8))
        pwq = ctx.enter_context(tc.tile_pool(name="pwq", bufs=8))
        pwk = ctx.enter_context(tc.tile_pool(name="pwk", bufs=8))
        pwv = ctx.enter_context(tc.tile_pool(name="pwv", bufs=8))
        pwo = ctx.enter_context(tc.tile_pool(name="pwo", bufs=2))
        pqt = ctx.enter_context(tc.tile_pool(name="pqt", bufs=4))
        pkt = ctx.enter_context(tc.tile_pool(name="pkt", bufs=4))
        pv = ctx.enter_context(tc.tile_pool(name="pv", bufs=16))
        pat = ctx.enter_context(tc.tile_pool(name="pat", bufs=4))
        ptmp = ctx.enter_context(tc.tile_pool(name="ptmp", bufs=4))
        pe_ = ctx.enter_context(tc.tile_pool(name="pe", bufs=3))
        prec = ctx.enter_context(tc.tile_pool(name="prec", bufs=2))
        pdram = ctx.enter_context(tc.tile_pool(name="pdram", bufs=4, space="DRAM"))
        psc = ctx.enter_context(tc.tile_pool(name="psc", bufs=2, space="PSUM"))
        ppv = ctx.enter_context(tc.tile_pool(name="ppv", bufs=4, space="PSUM"))

        # ---- constants ----
        cos2_sb = consts.tile([128, S], F32)
        nc.sync.dma_start(cos2_sb, d["cos2"])
        sin2_sb = consts.tile([128, S], F32)
        nc.sync.dma_start(sin2_sb, d["sin2"])
        mask_sb = consts.tile([128, 2, 128], F32)
        nc.sync.dma_start(mask_sb, d["mask2"])
        bq2_sb = consts.tile([128, 2], F32)
        nc.sync.dma_start(bq2_sb, d["bq2"])
        bk2_sb = consts.tile([128, 2], F32)
        nc.sync.dma_start(bk2_sb, d["bk2"])
        bvbc_sb = consts.tile([128, DH], F32)
        nc.sync.dma_start(bvbc_sb, d["bvbc"])

        # ---- weights ----
        wq_sb, wk_sb, wv_sb = [], [], []
        for e in range(NE):
            t = pwq.tile([128, DH], BF16, tag="wq")
            nc.sync.dma_start(t, d["wq_r"][e])
            wq_sb.append(t)
            t = pwk.tile([128, DH], BF16, tag="wk")
            nc.sync.dma_start(t, d["wk_r"][e])
            wk_sb.append(t)
            t = pwv.tile([128, DH], BF16, tag="wv")
            nc.sync.dma_start(t, d["wv_r"][e])
            wv_sb.append(t)
        wo_sb = []
        for p in range(2):
            t = pwo.tile([128, E], BF16, tag="wo")
            nc.sync.dma_start(t, d["wo_r"][p])
            wo_sb.append(t)

        # ---- x^T ----
        xt_sb = []
        for e in range(NE):
            t = px.tile([128, S], BF16, tag="xt")
            nc.sync.dma_start(t, d["xT_r"][e])
            xt_sb.append(t)

        # ---- emission helpers (interleaved so PE never starves while the
        # ---- scalar engine works through the softmax exps) ----
        qt_tiles, kt_tiles, at_tiles = {}, {}, {}
        v_sb = {}

        def emit_qk(w_sb, bias_sb, dst_pool, dst_tag, tiles, p, sp):
            ps = psc.tile([128, 2, 512], F32, tag="sc")
            for half in range(2):
                scol = (sp * 2 + half) * 512
                for e in range(NE):
                    nc.tensor.matmul(
                        ps[:, half, :],
                        w_sb[e][:, p * 128:(p + 1) * 128],
                        xt_sb[e][:, scol:scol + 512],
                        start=(e == 0), stop=(e == NE - 1),
                    )
            cols = slice(sp * 1024, (sp + 1) * 1024)
            tq = ptmp.tile([128, 1024], F32, tag="tmp")
            nc.scalar.activation(
                tq.rearrange("p (a b) -> p a b", b=512), ps,
                AF.Identity, bias=bias_sb[:, p:p + 1])
            tsh = ptmp.tile([128, 1024], F32, tag="tmp")
            nc.vector.stream_shuffle(tsh, tq, _SWAP_MASK)
            nc.vector.tensor_mul(tsh, tsh, sin2_sb[:, cols])
            nc.vector.tensor_mul(tq, tq, cos2_sb[:, cols])
            qt = dst_pool.tile([128, 1024], F32R, tag=dst_tag)
            nc.vector.tensor_add(qt, tq, tsh)
            tiles[(p, sp)] = qt

        def emit_v(st):
            psv = psc.tile([128, DH], F32, tag="sc")
            for e in range(NE):
                nc.tensor.matmul(
                    psv,
                    xt_sb[e][:, st * 128:(st + 1) * 128],
                    wv_sb[e],
                    start=(e == 0), stop=(e == NE - 1),
                )
            vt = pv.tile([128, HPG, 65], BF16, tag="v")
            nc.vector.memset(vt[:, :, 64:65], 1.0)
            nc.vector.tensor_add(
                vt[:, :, 0:64],
                psv.rearrange("p (h dd) -> p h dd", dd=64),
                bvbc_sb.rearrange("p (h dd) -> p h dd", dd=64),
            )
            v_sb[st] = vt

        def emit_attn(p, j):
            pvA = ppv.tile([128, 512], F32, tag="ppv")
            pvB = ppv.tile([128, 512], F32, tag="ppv")
            nkb = 4 * j + 4
            for kb in range(nkb):
                m = kb - 4 * j
                c0 = 128 * m if m > 0 else 0
                kt = kt_tiles[(p, kb // 8)]
                kcols = slice((kb % 8) * 128, (kb % 8) * 128 + 128)
                qt = qt_tiles[(p, j // 2)]
                qcols = slice((j % 2) * 512 + c0, (j % 2) * 512 + 512)
                sc = psc.tile([128, 2, 512], F32, tag="sc")
                nc.tensor.matmul(
                    sc[:, 0, c0:512],
                    kt[0:64, kcols],
                    qt[0:64, qcols],
                    start=True, stop=True, tile_position=(0, 0),
                )
                nc.tensor.matmul(
                    sc[:, 1, c0:512],
                    kt[64:128, kcols],
                    qt[64:128, qcols],
                    start=True, stop=True, tile_position=(64, 0),
                )
                if m >= 0:
                    nc.vector.tensor_add(
                        sc[:, :, c0:c0 + 128], sc[:, :, c0:c0 + 128], mask_sb)
                et = pe_.tile([128, 2, 512], BF16, tag="e")
                nc.scalar.activation(
                    et[:, :, c0:512], sc[:, :, c0:512], AF.Exp, scale=0.125)
                hA, hB = 2 * p, 2 * p + 1
                nc.tensor.matmul(
                    pvA[0:65, c0:512], v_sb[kb][:, hA, :], et[:, 0, c0:512],
                    start=(kb == 0), stop=(kb == nkb - 1),
                )
                nc.tensor.matmul(
                    pvB[0:65, c0:512], v_sb[kb][:, hB, :], et[:, 1, c0:512],
                    start=(kb == 0), stop=(kb == nkb - 1),
                )
            den = prec.tile([1, 1024], F32, tag="den")
            nc.vector.tensor_copy(den[:, 0:512], pvA[64:65, :])
            nc.vector.tensor_copy(den[:, 512:1024], pvB[64:65, :])
            rec = prec.tile([1, 1024], F32, tag="rec")
            nc.vector.reciprocal_approx_fast(rec, den)
            rec_d = pdram.tile([1, 1024], F32, tag="recd")
            nc.sync.dma_start(rec_d, rec)
            bcs = ptmp.tile([128, 512], F32, tag="tmp")
            nc.sync.dma_start(bcs[0:64, :], rec_d[:, 0:512].to_broadcast([64, 512]))
            nc.sync.dma_start(bcs[64:128, :], rec_d[:, 512:1024].to_broadcast([64, 512]))
            at = pat.tile([128, 512], BF16, tag="at")
            nc.vector.tensor_mul(at[0:64], pvA[0:64], bcs[0:64])
            nc.vector.tensor_mul(at[64:128], pvB[0:64], bcs[64:128])
            at_tiles[(p, j)] = at

        def emit_outproj(j):
            for et_i in range(NE):
                pso = ppv.tile([128, 512], F32, tag="ppv")
                for p in range(2):
                    nc.tensor.matmul(
                        pso,
                        wo_sb[p][:, et_i * 128:(et_i + 1) * 128],
                        at_tiles[(p, j)],
                        start=(p == 0), stop=(p == 1),
                    )
                stg = ptmp.tile([128, 1024], F32, tag="tmp")
                nc.vector.tensor_copy(stg[:, 0:512], pso)
                nc.sync.dma_start(
                    d["out_r"][et_i][:, j * 512:(j + 1) * 512], stg[:, 0:512])

        # ---- interleaved schedule ----
        emit_qk(wq_sb, bq2_sb, pqt, "qt", qt_tiles, 0, 0)
        emit_qk(wk_sb, bk2_sb, pkt, "kt", kt_tiles, 0, 0)
        for st in range(0, 4):
            emit_v(st)
        emit_attn(0, 0)
        for st in range(4, 8):
            emit_v(st)
        emit_qk(wq_sb, bq2_sb, pqt, "qt", qt_tiles, 0, 1)
        emit_qk(wk_sb, bk2_sb, pkt, "kt", kt_tiles, 0, 1)
        emit_qk(wq_sb, bq2_sb, pqt, "qt", qt_tiles, 1, 0)
        emit_qk(wk_sb, bk2_sb, pkt, "kt", kt_tiles, 1, 0)
        emit_attn(1, 0)
        emit_outproj(0)
        for st in range(8, 12):
            emit_v(st)
        emit_qk(wq_sb, bq2_sb, pqt, "qt", qt_tiles, 1, 1)
        emit_qk(wk_sb, bk2_sb, pkt, "kt", kt_tiles, 1, 1)
        emit_attn(0, 1)
        emit_attn(1, 1)
        emit_outproj(1)
        for st in range(12, 16):
            emit_v(st)
        emit_attn(0, 2)
        emit_attn(1, 2)
        emit_outproj(2)
        emit_attn(0, 3)
        emit_attn(1, 3)
        emit_outproj(3)


def make_host_inputs(x, Wq, bq, Wk, bk, Wv, bv, Wo, bo):
    """Shard + pre-transpose inputs per core. Returns (in_maps, bo)."""
    x = np.asarray(x, np.float32)
    Wq, Wk, Wv, Wo = (np.asarray(w, np.float32) for w in (Wq, Wk, Wv, Wo))
    bq, bk, bv, bo = (np.asarray(b_, np.float32) for b_ in (bq, bk, bv, bo))

    # RoPE tables
    half = D // 2
    inv_freq = 1.0 / (ROPE_BASE ** (np.arange(half, dtype=np.float64) / half))
    pos = np.arange(S, dtype=np.float64)
    sinus = pos[:, None] * inv_freq[None, :]           # [S, 32]
    sin_full = np.repeat(np.sin(sinus), 2, axis=1)     # [S, 64] interleave-dup
    cos_full = np.repeat(np.cos(sinus), 2, axis=1)
    sgn = np.where(np.arange(D) % 2 == 0, -1.0, 1.0)
    cos2 = np.tile(cos_full.T, (2, 1)).astype(np.float32)
    sin2 = np.tile((sin_full * sgn[None, :]).T, (2, 1)).astype(np.float32)

    tri = np.where(np.arange(128)[:, None] <= np.arange(128)[None, :],
                   0.0, MASK_VAL).astype(np.float32)
    mask2 = np.stack([tri, tri], axis=1)               # [128, 2, 128]
    xT = [np.ascontiguousarray(x[b_].T) for b_ in range(B)]
    in_maps = []
    for c in range(8):
        b_, hg = c // 4, c % 4
        rows = slice(DH * hg, DH * hg + DH)
        bf = ml_dtypes.bfloat16
        in_maps.append({
            "xT": xT[b_].astype(bf),
            "wqT": np.ascontiguousarray(Wq[rows].T).astype(bf),
            "wkT": np.ascontiguousarray(Wk[rows].T).astype(bf),
            "wvT": np.ascontiguousarray(Wv[rows].T).astype(bf),
            "woST": np.ascontiguousarray(Wo[:, rows].T).astype(bf),
            "bq2": np.ascontiguousarray(bq[rows].reshape(2, 128).T),
            "bk2": np.ascontiguousarray(bk[rows].reshape(2, 128).T),
            "bvbc": np.tile(bv[rows][None, :], (128, 1)).astype(np.float32),
            "cos2": cos2,
            "sin2": sin2,
            "mask2": mask2,
        })
    return in_maps, bo


_NC_CACHE = {}


def get_nc():
    if "nc" not in _NC_CACHE:
        _NC_CACHE["nc"] = build_nc()
    return _NC_CACHE["nc"]


def kernel(**inputs):
    in_maps, bo = make_host_inputs(**inputs)
    nc = get_nc()
    res = run_bass_kernel_spmd(nc, in_maps, core_ids=list(range(8)))
    out = np.zeros((B, S, E), np.float32)
    for c in range(8):
        out[c // 4] += res.results[c]["out"].T
    out += bo[None, None, :]
    return out


# revision 24
# speedup vs baseline: 1.4280x; 1.1184x over previous
"""Causal RoPE self-attention, distributed over 8 TRN2 NeuronCores.

Sharding: batch (2) x head-groups (4 heads each) -> 8 cores.
Each core computes, for its (batch b, head-group hg):
    q/k/v projections for its 4 heads (tensor-parallel column split),
    RoPE, causal attention, and the row-parallel slice of the output
    projection, producing a partial output partialT = WoS^T @ attnT
    of shape [E, S].  The host sums the 4 partials per batch and adds bo.

On-device layout notes:
  - activations live transposed: qT/kT are [head-dim, seq] so the
    score matmul sT[k, q] = K Q^T contracts over d on partitions, and
    softmax denominators come from an extra all-ones column in V.
  - matmuls run with float32r operand views (full fp32 storage,
    1 cycle/row TensorEngine rate); exp'd scores and V are bf16.
"""

import ml_dtypes
import numpy as np

import concourse.bass as bass
import concourse.tile as tile
from concourse import bacc, mybir
from concourse.bass_utils import run_bass_kernel_spmd

F32 = mybir.dt.float32
F32R = mybir.dt.float32r
BF16 = mybir.dt.bfloat16
AF = mybir.ActivationFunctionType

B, S, E = 2, 2048, 1024
H, D = 16, 64
HPG = 4                # heads per core
DH = HPG * D           # 256 head-dims per core
NE = E // 128          # 8 e-chunks
NST = S // 128         # 16 s-tiles / key blocks
NSL = S // 512         # 4 q-slices
ROPE_BASE = 10000.0
MASK_VAL = -30000.0

_SWAP_MASK = [i ^ 1 for i in range(32)]


def build_nc():
    """Build + compile the per-core Bass graph (same graph on all 8 cores)."""
    nc = bacc.Bacc("TRN2", target_bir_lowering=False, debug=False, num_devices=8)

    def din(name, shape, dt=F32):
        return nc.dram_tensor(name, shape, dt, kind="ExternalInput").ap()

    xT = din("xT", [E, S], BF16)
    wqT = din("wqT", [E, DH], BF16)
    wkT = din("wkT", [E, DH], BF16)
    wvT = din("wvT", [E, DH], BF16)
    woST = din("woST", [DH, E], BF16)
    bq2 = din("bq2", [128, 2])
    bk2 = din("bk2", [128, 2])
    bvbc = din("bvbc", [128, DH])
    cos2 = din("cos2", [128, S], BF16)      # cosT duplicated on both halves
    sin2 = din("sin2", [128, S], BF16)      # signed sinT duplicated on both halves
    mask2 = din("mask2", [128, 2, 128])     # causal tri mask, duplicated x2
    out = nc.dram_tensor("out", [E, S], F32, kind="ExternalOutput").ap()

    xT_r = xT.rearrange("(n p) s -> n p s", p=128)
    wq_r = wqT.rearrange("(n p) d -> n p d", p=128)
    wk_r = wkT.rearrange("(n p) d -> n p d", p=128)
    wv_r = wvT.rearrange("(n p) d -> n p d", p=128)
    wo_r = woST.rearrange("(n p) e -> n p e", p=128)
    out_r = out.rearrange("(n p) s -> n p s", p=128)

    with tile.TileContext(nc) as tc, nc.allow_low_precision(
            reason="float32r matmul operands (fp32 storage, TF32-rate)"):
        _emit(tc, nc, dict(
            xT_r=xT_r, wq_r=wq_r, wk_r=wk_r, wv_r=wv_r, wo_r=wo_r, out_r=out_r,
            bq2=bq2, bk2=bk2, bvbc=bvbc, cos2=cos2, sin2=sin2, mask2=mask2,
        ))
    nc.compile()
    return nc


def _emit(tc, nc, d):
    from contextlib import ExitStack
    ctx = ExitStack()
    with ctx:
        consts = ctx.enter_context(tc.tile_pool(name="consts", bufs=1))
        px = ctx.enter_context(tc.tile_pool(name="px", bufs=8))
        pwq = ctx.enter_context(tc.tile_pool(name="pwq", bufs=8))
        pwk = ctx.enter_context(tc.tile_pool(name="pwk", bufs=8))
        pwv = ctx.enter_context(tc.tile_pool(name="pwv", bufs=8))
        pwo = ctx.enter_context(tc.tile_pool(name="pwo", bufs=2))
        pqt = ctx.enter_context(tc.tile_pool(name="pqt", bufs=4))
        pkt = ctx.enter_context(tc.tile_pool(name="pkt", bufs=4))
        pv = ctx.enter_context(tc.tile_pool(name="pv", bufs=16))
        pat = ctx.enter_context(tc.tile_pool(name="pat", bufs=4))
        ptmp = ctx.enter_context(tc.tile_pool(name="ptmp", bufs=4))
        pe_ = ctx.enter_context(tc.tile_pool(name="pe", bufs=3))
        prec = ctx.enter_context(tc.tile_pool(name="prec", bufs=2))
        pdram = ctx.enter_context(tc.tile_pool(name="pdram", bufs=4, space="DRAM"))
        psc = ctx.enter_context(tc.tile_pool(name="psc", bufs=2, space="PSUM"))
        ppv = ctx.enter_context(tc.tile_pool(name="ppv", bufs=4, space="PSUM"))

        # ---- constants ----
        cos2_sb = consts.tile([128, S], BF16)
        nc.sync.dma_start(cos2_sb, d["cos2"])
        sin2_sb = consts.tile([128, S], BF16)
        nc.sync.dma_start(sin2_sb, d["sin2"])
        mask_sb = consts.tile([128, 2, 128], F32)
        nc.sync.dma_start(mask_sb, d["mask2"])
        bq2_sb = consts.tile([128, 2], F32)
        nc.sync.dma_start(bq2_sb, d["bq2"])
        bk2_sb = consts.tile([128, 2], F32)
        nc.sync.dma_start(bk2_sb, d["bk2"])
        bvbc_sb = consts.tile([128, DH], F32)
        nc.sync.dma_start(bvbc_sb, d["bvbc"])

        # ---- weights ----
        wq_sb, wk_sb, wv_sb = [], [], []
        for e in range(NE):
            t = pwq.tile([128, DH], BF16, tag="wq")
            nc.sync.dma_start(t, d["wq_r"][e])
            wq_sb.append(t)
            t = pwk.tile([128, DH], BF16, tag="wk")
            nc.sync.dma_start(t, d["wk_r"][e])
            wk_sb.append(t)
            t = pwv.tile([128, DH], BF16, tag="wv")
            nc.sync.dma_start(t, d["wv_r"][e])
            wv_sb.append(t)
        wo_sb = []
        for p in range(2):
            t = pwo.tile([128, E], BF16, tag="wo")
            nc.sync.dma_start(t, d["wo_r"][p])
            wo_sb.append(t)

        # ---- x^T ----
        xt_sb = []
        for e in range(NE):
            t = px.tile([128, S], BF16, tag="xt")
            nc.sync.dma_start(t, d["xT_r"][e])
            xt_sb.append(t)

        # ---- emission helpers (interleaved so PE never starves while the
        # ---- scalar engine works through the softmax exps) ----
        qt_tiles, kt_tiles, at_tiles = {}, {}, {}
        v_sb = {}

        def emit_qk(w_sb, bias_sb, dst_pool, dst_tag, tiles, p, sp):
            ps = psc.tile([128, 2, 512], F32, tag="sc")
            for half in range(2):
                scol = (sp * 2 + half) * 512
                for e in range(NE):
                    nc.tensor.matmul(
                        ps[:, half, :],
                        w_sb[e][:, p * 128:(p + 1) * 128],
                        xt_sb[e][:, scol:scol + 512],
                        start=(e == 0), stop=(e == NE - 1),
                    )
            cols = slice(sp * 1024, (sp + 1) * 1024)
            tq = ptmp.tile([128, 1024], BF16, tag="tmpb")
            nc.scalar.activation(
                tq.rearrange("p (a b) -> p a b", b=512), ps,
                AF.Identity, bias=bias_sb[:, p:p + 1])
            tsh = ptmp.tile([128, 1024], BF16, tag="tmpb")
            nc.vector.stream_shuffle(tsh, tq, _SWAP_MASK)
            nc.vector.tensor_mul(tsh, tsh, sin2_sb[:, cols])
            nc.vector.tensor_mul(tq, tq, cos2_sb[:, cols])
            qt = dst_pool.tile([128, 1024], BF16, tag=dst_tag)
            nc.vector.tensor_add(qt, tq, tsh)
            tiles[(p, sp)] = qt

        def emit_v(st):
            psv = psc.tile([128, DH], F32, tag="sc")
            for e in range(NE):
                nc.tensor.matmul(
                    psv,
                    xt_sb[e][:, st * 128:(st + 1) * 128],
                    wv_sb[e],
                    start=(e == 0), stop=(e == NE - 1),
                )
            vt = pv.tile([128, HPG, 65], BF16, tag="v")
            nc.vector.memset(vt[:, :, 64:65], 1.0)
            nc.vector.tensor_add(
                vt[:, :, 0:64],
                psv.rearrange("p (h dd) -> p h dd", dd=64),
                bvbc_sb.rearrange("p (h dd) -> p h dd", dd=64),
            )
            v_sb[st] = vt

        def emit_attn(p, j):
            pvA = ppv.tile([128, 512], F32, tag="ppv")
            pvB = ppv.tile([128, 512], F32, tag="ppv")
            nkb = 4 * j + 4
            for kb in range(nkb):
                m = kb - 4 * j
                c0 = 128 * m if m > 0 else 0
                kt = kt_tiles[(p, kb // 8)]
                kcols = slice((kb % 8) * 128, (kb % 8) * 128 + 128)
                qt = qt_tiles[(p, j // 2)]
                qcols = slice((j % 2) * 512 + c0, (j % 2) * 512 + 512)
                sc = psc.tile([128, 2, 512], F32, tag="sc")
                nc.tensor.matmul(
                    sc[:, 0, c0:512],
                    kt[0:64, kcols],
                    qt[0:64, qcols],
                    start=True, stop=True, tile_position=(0, 0),
                )
                nc.tensor.matmul(
                    sc[:, 1, c0:512],
                    kt[64:128, kcols],
                    qt[64:128, qcols],
                    start=True, stop=True, tile_position=(64, 0),
                )
                if m >= 0:
                    nc.vector.tensor_add(
                        sc[:, :, c0:c0 + 128], sc[:, :, c0:c0 + 128], mask_sb)
                et = pe_.tile([128, 2, 512], BF16, tag="e")
                nc.scalar.activation(
                    et[:, :, c0:512], sc[:, :, c0:512], AF.Exp, scale=0.125)
                hA, hB = 2 * p, 2 * p + 1
                nc.tensor.matmul(
                    pvA[0:65, c0:512], v_sb[kb][:, hA, :], et[:, 0, c0:512],
                    start=(kb == 0), stop=(kb == nkb - 1),
                )
                nc.tensor.matmul(
                    pvB[0:65, c0:512], v_sb[kb][:, hB, :], et[:, 1, c0:512],
                    start=(kb == 0), stop=(kb == nkb - 1),
                )
            den = prec.tile([1, 1024], F32, tag="den")
            nc.vector.tensor_copy(den[:, 0:512], pvA[64:65, :])
            nc.vector.tensor_copy(den[:, 512:1024], pvB[64:65, :])
            rec = prec.tile([1, 1024], F32, tag="rec")
            nc.vector.reciprocal_approx_fast(rec, den)
            rec_d = pdram.tile([1, 1024], F32, tag="recd")
            nc.sync.dma_start(rec_d, rec)
            bcs = ptmp.tile([128, 512], F32, tag="tmp")
            nc.sync.dma_start(bcs[0:64, :], rec_d[:, 0:512].to_broadcast([64, 512]))
            nc.sync.dma_start(bcs[64:128, :], rec_d[:, 512:1024].to_broadcast([64, 512]))
            at = pat.tile([128, 512], BF16, tag="at")
            nc.vector.tensor_mul(at[0:64], pvA[0:64], bcs[0:64])
            nc.vector.tensor_mul(at[64:128], pvB[0:64], bcs[64:128])
            at_tiles[(p, j)] = at

        def emit_outproj(j):
            for et_i in range(NE):
                pso = ppv.tile([128, 512], F32, tag="ppv")
                for p in range(2):
                    nc.tensor.matmul(
                        pso,
                        wo_sb[p][:, et_i * 128:(et_i + 1) * 128],
                        at_tiles[(p, j)],
                        start=(p == 0), stop=(p == 1),
                    )
                stg = ptmp.tile([128, 1024], F32, tag="tmp")
                nc.vector.tensor_copy(stg[:, 0:512], pso)
                nc.sync.dma_start(
                    d["out_r"][et_i][:, j * 512:(j + 1) * 512], stg[:, 0:512])

        # ---- interleaved schedule ----
        emit_qk(wq_sb, bq2_sb, pqt, "qt", qt_tiles, 0, 0)
        emit_qk(wk_sb, bk2_sb, pkt, "kt", kt_tiles, 0, 0)
        for st in range(0, 4):
            emit_v(st)
        emit_attn(0, 0)
        for st in range(4, 8):
            emit_v(st)
        emit_qk(wq_sb, bq2_sb, pqt, "qt", qt_tiles, 0, 1)
        emit_qk(wk_sb, bk2_sb, pkt, "kt", kt_tiles, 0, 1)
        emit_qk(wq_sb, bq2_sb, pqt, "qt", qt_tiles, 1, 0)
        emit_qk(wk_sb, bk2_sb, pkt, "kt", kt_tiles, 1, 0)
        emit_attn(1, 0)
        emit_outproj(0)
        for st in range(8, 12):
            emit_v(st)
        emit_qk(wq_sb, bq2_sb, pqt, "qt", qt_tiles, 1, 1)
        emit_qk(wk_sb, bk2_sb, pkt, "kt", kt_tiles, 1, 1)
        emit_attn(0, 1)
        emit_attn(1, 1)
        emit_outproj(1)
        for st in range(12, 16):
            emit_v(st)
        emit_attn(0, 2)
        emit_attn(1, 2)
        emit_outproj(2)
        emit_attn(0, 3)
        emit_attn(1, 3)
        emit_outproj(3)


def make_host_inputs(x, Wq, bq, Wk, bk, Wv, bv, Wo, bo):
    """Shard + pre-transpose inputs per core. Returns (in_maps, bo)."""
    x = np.asarray(x, np.float32)
    Wq, Wk, Wv, Wo = (np.asarray(w, np.float32) for w in (Wq, Wk, Wv, Wo))
    bq, bk, bv, bo = (np.asarray(b_, np.float32) for b_ in (bq, bk, bv, bo))

    # RoPE tables
    half = D // 2
    inv_freq = 1.0 / (ROPE_BASE ** (np.arange(half, dtype=np.float64) / half))
    pos = np.arange(S, dtype=np.float64)
    sinus = pos[:, None] * inv_freq[None, :]           # [S, 32]
    sin_full = np.repeat(np.sin(sinus), 2, axis=1)     # [S, 64] interleave-dup
    cos_full = np.repeat(np.cos(sinus), 2, axis=1)
    sgn = np.where(np.arange(D) % 2 == 0, -1.0, 1.0)
    cos2 = np.tile(cos_full.T, (2, 1)).astype(ml_dtypes.bfloat16)
    sin2 = np.tile((sin_full * sgn[None, :]).T, (2, 1)).astype(ml_dtypes.bfloat16)

    tri = np.where(np.arange(128)[:, None] <= np.arange(128)[None, :],
                   0.0, MASK_VAL).astype(np.float32)
    mask2 = np.stack([tri, tri], axis=1)               # [128, 2, 128]
    xT = [np.ascontiguousarray(x[b_].T) for b_ in range(B)]
    in_maps = []
    for c in range(8):
        b_, hg = c // 4, c % 4
        rows = slice(DH * hg, DH * hg + DH)
        bf = ml_dtypes.bfloat16
        in_maps.append({
            "xT": xT[b_].astype(bf),
            "wqT": np.ascontiguousarray(Wq[rows].T).astype(bf),
            "wkT": np.ascontiguousarray(Wk[rows].T).astype(bf),
            "wvT": np.ascontiguousarray(Wv[rows].T).astype(bf),
            "woST": np.ascontiguousarray(Wo[:, rows].T).astype(bf),
            "bq2": np.ascontiguousarray(bq[rows].reshape(2, 128).T),
            "bk2": np.ascontiguousarray(bk[rows].reshape(2, 128).T),
            "bvbc": np.tile(bv[rows][None, :], (128, 1)).astype(np.float32),
            "cos2": cos2,
            "sin2": sin2,
            "mask2": mask2,
        })
    return in_maps, bo


_NC_CACHE = {}


def get_nc():
    if "nc" not in _NC_CACHE:
        _NC_CACHE["nc"] = build_nc()
    return _NC_CACHE["nc"]


def kernel(**inputs):
    in_maps, bo = make_host_inputs(**inputs)
    nc = get_nc()
    res = run_bass_kernel_spmd(nc, in_maps, core_ids=list(range(8)))
    out = np.zeros((B, S, E), np.float32)
    for c in range(8):
        out[c // 4] += res.results[c]["out"].T
    out += bo[None, None, :]
    return out


# revision 25
# speedup vs baseline: 1.5466x; 1.0831x over previous
"""Causal RoPE self-attention, distributed over 8 TRN2 NeuronCores.

Sharding: batch (2) x head-groups (4 heads each) -> 8 cores.
Each core computes, for its (batch b, head-group hg):
    q/k/v projections for its 4 heads (tensor-parallel column split),
    RoPE, causal attention, and the row-parallel slice of the output
    projection, producing a partial output partialT = WoS^T @ attnT
    of shape [E, S].  The host sums the 4 partials per batch and adds bo.

On-device layout notes:
  - activations live transposed: qT/kT are [head-dim, seq] so the
    score matmul sT[k, q] = K Q^T contracts over d on partitions, and
    softmax denominators come from an extra all-ones column in V.
  - matmuls run with float32r operand views (full fp32 storage,
    1 cycle/row TensorEngine rate); exp'd scores and V are bf16.
"""

import ml_dtypes
import numpy as np

import concourse.bass as bass
import concourse.tile as tile
from concourse import bacc, mybir
from concourse.bass_utils import run_bass_kernel_spmd

F32 = mybir.dt.float32
F32R = mybir.dt.float32r
BF16 = mybir.dt.bfloat16
AF = mybir.ActivationFunctionType

B, S, E = 2, 2048, 1024
H, D = 16, 64
HPG = 4                # heads per core
DH = HPG * D           # 256 head-dims per core
NE = E // 128          # 8 e-chunks
NST = S // 128         # 16 s-tiles / key blocks
NSL = S // 512         # 4 q-slices
ROPE_BASE = 10000.0
MASK_VAL = -30000.0

_SWAP_MASK = [i ^ 1 for i in range(32)]


def build_nc():
    """Build + compile the per-core Bass graph (same graph on all 8 cores)."""
    nc = bacc.Bacc("TRN2", target_bir_lowering=False, debug=False, num_devices=8)

    def din(name, shape, dt=F32):
        return nc.dram_tensor(name, shape, dt, kind="ExternalInput").ap()

    xT = din("xT", [E, S], BF16)
    wqT = din("wqT", [E, DH], BF16)
    wkT = din("wkT", [E, DH], BF16)
    wvT = din("wvT", [E, DH], BF16)
    woST = din("woST", [DH, E], BF16)
    bq2 = din("bq2", [128, 2])
    bk2 = din("bk2", [128, 2])
    bvbc = din("bvbc", [128, DH])
    cos2 = din("cos2", [128, S], BF16)      # cosT duplicated on both halves
    sin2 = din("sin2", [128, S], BF16)      # signed sinT duplicated on both halves
    mask2 = din("mask2", [128, 2, 128])     # causal tri mask, duplicated x2
    out = nc.dram_tensor("out", [E, S], F32, kind="ExternalOutput").ap()

    xT_r = xT.rearrange("(n p) s -> n p s", p=128)
    wq_r = wqT.rearrange("(n p) d -> n p d", p=128)
    wk_r = wkT.rearrange("(n p) d -> n p d", p=128)
    wv_r = wvT.rearrange("(n p) d -> n p d", p=128)
    wo_r = woST.rearrange("(n p) e -> n p e", p=128)
    out_r = out.rearrange("(n p) s -> n p s", p=128)

    with tile.TileContext(nc) as tc, nc.allow_low_precision(
            reason="float32r matmul operands (fp32 storage, TF32-rate)"):
        _emit(tc, nc, dict(
            xT_r=xT_r, wq_r=wq_r, wk_r=wk_r, wv_r=wv_r, wo_r=wo_r, out_r=out_r,
            bq2=bq2, bk2=bk2, bvbc=bvbc, cos2=cos2, sin2=sin2, mask2=mask2,
        ))
    nc.compile()
    return nc


def _emit(tc, nc, d):
    from contextlib import ExitStack
    ctx = ExitStack()
    with ctx:
        consts = ctx.enter_context(tc.tile_pool(name="consts", bufs=1))
        px = ctx.enter_context(tc.tile_pool(name="px", bufs=8))
        pwq = ctx.enter_context(tc.tile_pool(name="pwq", bufs=8))
        pwk = ctx.enter_context(tc.tile_pool(name="pwk", bufs=8))
        pwv = ctx.enter_context(tc.tile_pool(name="pwv", bufs=8))
        pwo = ctx.enter_context(tc.tile_pool(name="pwo", bufs=2))
        pqt = ctx.enter_context(tc.tile_pool(name="pqt", bufs=4))
        pkt = ctx.enter_context(tc.tile_pool(name="pkt", bufs=4))
        pv = ctx.enter_context(tc.tile_pool(name="pv", bufs=16))
        pat = ctx.enter_context(tc.tile_pool(name="pat", bufs=4))
        ptmp = ctx.enter_context(tc.tile_pool(name="ptmp", bufs=4))
        pe_ = ctx.enter_context(tc.tile_pool(name="pe", bufs=4))
        prec = ctx.enter_context(tc.tile_pool(name="prec", bufs=2))
        pdram = ctx.enter_context(tc.tile_pool(name="pdram", bufs=4, space="DRAM"))
        psc = ctx.enter_context(tc.tile_pool(name="psc", bufs=2, space="PSUM"))
        ppv = ctx.enter_context(tc.tile_pool(name="ppv", bufs=4, space="PSUM"))

        # ---- input DMAs: x chunks + q-weights first so the first
        # ---- projection's accumulation can start as data streams in ----
        wq_sb, wk_sb, wv_sb, wo_sb = [], [], [], []
        xt_sb = []
        for e in range(NE):
            t = pwq.tile([128, DH], BF16, tag="wq")
            nc.sync.dma_start(t, d["wq_r"][e])
            wq_sb.append(t)
            t = px.tile([128, S], BF16, tag="xt")
            nc.sync.dma_start(t, d["xT_r"][e])
            xt_sb.append(t)
        for e in range(NE):
            t = pwk.tile([128, DH], BF16, tag="wk")
            nc.sync.dma_start(t, d["wk_r"][e])
            wk_sb.append(t)
            t = pwv.tile([128, DH], BF16, tag="wv")
            nc.sync.dma_start(t, d["wv_r"][e])
            wv_sb.append(t)
        cos2_sb = consts.tile([128, S], BF16)
        nc.sync.dma_start(cos2_sb, d["cos2"])
        sin2_sb = consts.tile([128, S], BF16)
        nc.sync.dma_start(sin2_sb, d["sin2"])
        mask_sb = consts.tile([128, 2, 128], F32)
        nc.sync.dma_start(mask_sb, d["mask2"])
        bq2_sb = consts.tile([128, 2], F32)
        nc.sync.dma_start(bq2_sb, d["bq2"])
        bk2_sb = consts.tile([128, 2], F32)
        nc.sync.dma_start(bk2_sb, d["bk2"])
        bvbc_sb = consts.tile([128, DH], F32)
        nc.sync.dma_start(bvbc_sb, d["bvbc"])
        for p in range(2):
            t = pwo.tile([128, E], BF16, tag="wo")
            nc.sync.dma_start(t, d["wo_r"][p])
            wo_sb.append(t)

        # ---- emission helpers (interleaved so PE never starves while the
        # ---- scalar engine works through the softmax exps) ----
        qt_tiles, kt_tiles, at_tiles = {}, {}, {}
        v_sb = {}

        def emit_qk(w_sb, bias_sb, dst_pool, dst_tag, tiles, p, sp):
            ps = psc.tile([128, 2, 512], F32, tag="sc")
            for half in range(2):
                scol = (sp * 2 + half) * 512
                for e in range(NE):
                    nc.tensor.matmul(
                        ps[:, half, :],
                        w_sb[e][:, p * 128:(p + 1) * 128],
                        xt_sb[e][:, scol:scol + 512],
                        start=(e == 0), stop=(e == NE - 1),
                    )
            cols = slice(sp * 1024, (sp + 1) * 1024)
            tq = ptmp.tile([128, 1024], BF16, tag="tmpb")
            nc.scalar.activation(
                tq.rearrange("p (a b) -> p a b", b=512), ps,
                AF.Identity, bias=bias_sb[:, p:p + 1])
            tsh = ptmp.tile([128, 1024], BF16, tag="tmpb")
            nc.vector.stream_shuffle(tsh, tq, _SWAP_MASK)
            nc.vector.tensor_mul(tsh, tsh, sin2_sb[:, cols])
            nc.vector.tensor_mul(tq, tq, cos2_sb[:, cols])
            qt = dst_pool.tile([128, 1024], BF16, tag=dst_tag)
            nc.vector.tensor_add(qt, tq, tsh)
            tiles[(p, sp)] = qt

        def emit_v(st):
            psv = psc.tile([128, DH], F32, tag="sc")
            for e in range(NE):
                nc.tensor.matmul(
                    psv,
                    xt_sb[e][:, st * 128:(st + 1) * 128],
                    wv_sb[e],
                    start=(e == 0), stop=(e == NE - 1),
                )
            vt = pv.tile([128, HPG, 65], BF16, tag="v")
            nc.vector.memset(vt[:, :, 64:65], 1.0)
            nc.vector.tensor_add(
                vt[:, :, 0:64],
                psv.rearrange("p (h dd) -> p h dd", dd=64),
                bvbc_sb.rearrange("p (h dd) -> p h dd", dd=64),
            )
            v_sb[st] = vt

        def emit_attn(p, j):
            pvA = ppv.tile([128, 512], F32, tag="ppv")
            pvB = ppv.tile([128, 512], F32, tag="ppv")
            nkb = 4 * j + 4
            for kb in range(nkb):
                m = kb - 4 * j
                c0 = 128 * m if m > 0 else 0
                kt = kt_tiles[(p, kb // 8)]
                kcols = slice((kb % 8) * 128, (kb % 8) * 128 + 128)
                qt = qt_tiles[(p, j // 2)]
                qcols = slice((j % 2) * 512 + c0, (j % 2) * 512 + 512)
                sc = psc.tile([128, 2, 512], F32, tag="sc")
                nc.tensor.matmul(
                    sc[:, 0, c0:512],
                    kt[0:64, kcols],
                    qt[0:64, qcols],
                    start=True, stop=True, tile_position=(0, 0),
                )
                nc.tensor.matmul(
                    sc[:, 1, c0:512],
                    kt[64:128, kcols],
                    qt[64:128, qcols],
                    start=True, stop=True, tile_position=(64, 0),
                )
                et = pe_.tile([128, 2, 512], BF16, tag="e")
                nc.scalar.activation(
                    et[:, :, c0:512], sc[:, :, c0:512], AF.Exp, scale=0.125)
                if m >= 0:
                    # zero e where key > query inside the diagonal block
                    nc.gpsimd.affine_select(
                        out=et[:, :, c0:c0 + 128],
                        in_=et[:, :, c0:c0 + 128],
                        compare_op=mybir.AluOpType.is_ge,
                        fill=0.0,
                        base=0,
                        pattern=[[0, 2], [1, 128]],
                        channel_multiplier=-1,
                    )
                hA, hB = 2 * p, 2 * p + 1
                nc.tensor.matmul(
                    pvA[0:65, c0:512], v_sb[kb][:, hA, :], et[:, 0, c0:512],
                    start=(kb == 0), stop=(kb == nkb - 1),
                )
                nc.tensor.matmul(
                    pvB[0:65, c0:512], v_sb[kb][:, hB, :], et[:, 1, c0:512],
                    start=(kb == 0), stop=(kb == nkb - 1),
                )
            den = prec.tile([1, 1024], F32, tag="den")
            nc.vector.tensor_copy(den[:, 0:512], pvA[64:65, :])
            nc.vector.tensor_copy(den[:, 512:1024], pvB[64:65, :])
            rec = prec.tile([1, 1024], F32, tag="rec")
            nc.vector.reciprocal_approx_fast(rec, den)
            rec_d = pdram.tile([1, 1024], F32, tag="recd")
            nc.sync.dma_start(rec_d, rec)
            bcs = ptmp.tile([128, 512], F32, tag="tmp")
            nc.sync.dma_start(bcs[0:64, :], rec_d[:, 0:512].to_broadcast([64, 512]))
            nc.sync.dma_start(bcs[64:128, :], rec_d[:, 512:1024].to_broadcast([64, 512]))
            at = pat.tile([128, 512], BF16, tag="at")
            nc.vector.tensor_mul(at[0:64], pvA[0:64], bcs[0:64])
            nc.vector.tensor_mul(at[64:128], pvB[0:64], bcs[64:128])
            at_tiles[(p, j)] = at

        def emit_outproj(j):
            for et_i in range(NE):
                pso = ppv.tile([128, 512], F32, tag="ppv")
                for p in range(2):
                    nc.tensor.matmul(
                        pso,
                        wo_sb[p][:, et_i * 128:(et_i + 1) * 128],
                        at_tiles[(p, j)],
                        start=(p == 0), stop=(p == 1),
                    )
                stg = ptmp.tile([128, 1024], F32, tag="tmp")
                nc.vector.tensor_copy(stg[:, 0:512], pso)
                nc.sync.dma_start(
                    d["out_r"][et_i][:, j * 512:(j + 1) * 512], stg[:, 0:512])

        # ---- interleaved schedule ----
        emit_qk(wq_sb, bq2_sb, pqt, "qt", qt_tiles, 0, 0)
        emit_qk(wk_sb, bk2_sb, pkt, "kt", kt_tiles, 0, 0)
        for st in range(0, 4):
            emit_v(st)
        emit_attn(0, 0)
        for st in range(4, 8):
            emit_v(st)
        emit_qk(wq_sb, bq2_sb, pqt, "qt", qt_tiles, 0, 1)
        emit_qk(wk_sb, bk2_sb, pkt, "kt", kt_tiles, 0, 1)
        emit_qk(wq_sb, bq2_sb, pqt, "qt", qt_tiles, 1, 0)
        emit_qk(wk_sb, bk2_sb, pkt, "kt", kt_tiles, 1, 0)
        emit_attn(1, 0)
        emit_outproj(0)
        for st in range(8, 12):
            emit_v(st)
        emit_qk(wq_sb, bq2_sb, pqt, "qt", qt_tiles, 1, 1)
        emit_qk(wk_sb, bk2_sb, pkt, "kt", kt_tiles, 1, 1)
        emit_attn(0, 1)
        emit_attn(1, 1)
        emit_outproj(1)
        for st in range(12, 16):
            emit_v(st)
        emit_attn(0, 2)
        emit_attn(1, 2)
        emit_outproj(2)
        emit_attn(0, 3)
        emit_attn(1, 3)
        emit_outproj(3)


def make_host_inputs(x, Wq, bq, Wk, bk, Wv, bv, Wo, bo):
    """Shard + pre-transpose inputs per core. Returns (in_maps, bo)."""
    x = np.asarray(x, np.float32)
    Wq, Wk, Wv, Wo = (np.asarray(w, np.float32) for w in (Wq, Wk, Wv, Wo))
    bq, bk, bv, bo = (np.asarray(b_, np.float32) for b_ in (bq, bk, bv, bo))

    # RoPE tables
    half = D // 2
    inv_freq = 1.0 / (ROPE_BASE ** (np.arange(half, dtype=np.float64) / half))
    pos = np.arange(S, dtype=np.float64)
    sinus = pos[:, None] * inv_freq[None, :]           # [S, 32]
    sin_full = np.repeat(np.sin(sinus), 2, axis=1)     # [S, 64] interleave-dup
    cos_full = np.repeat(np.cos(sinus), 2, axis=1)
    sgn = np.where(np.arange(D) % 2 == 0, -1.0, 1.0)
    cos2 = np.tile(cos_full.T, (2, 1)).astype(ml_dtypes.bfloat16)
    sin2 = np.tile((sin_full * sgn[None, :]).T, (2, 1)).astype(ml_dtypes.bfloat16)

    tri = np.where(np.arange(128)[:, None] <= np.arange(128)[None, :],
                   0.0, MASK_VAL).astype(np.float32)
    mask2 = np.stack([tri, tri], axis=1)               # [128, 2, 128]
    xT = [np.ascontiguousarray(x[b_].T) for b_ in range(B)]
    in_maps = []
    for c in range(8):
        b_, hg = c // 4, c % 4
        rows = slice(DH * hg, DH * hg + DH)
        bf = ml_dtypes.bfloat16
        in_maps.append({
            "xT": xT[b_].astype(bf),
            "wqT": np.ascontiguousarray(Wq[rows].T).astype(bf),
            "wkT": np.ascontiguousarray(Wk[rows].T).astype(bf),
            "wvT": np.ascontiguousarray(Wv[rows].T).astype(bf),
            "woST": np.ascontiguousarray(Wo[:, rows].T).astype(bf),
            "bq2": np.ascontiguousarray(bq[rows].reshape(2, 128).T),
            "bk2": np.ascontiguousarray(bk[rows].reshape(2, 128).T),
            "bvbc": np.tile(bv[rows][None, :], (128, 1)).astype(np.float32),
            "cos2": cos2,
            "sin2": sin2,
            "mask2": mask2,
        })
    return in_maps, bo


_NC_CACHE = {}


def get_nc():
    if "nc" not in _NC_CACHE:
        _NC_CACHE["nc"] = build_nc()
    return _NC_CACHE["nc"]


def kernel(**inputs):
    in_maps, bo = make_host_inputs(**inputs)
    nc = get_nc()
    res = run_bass_kernel_spmd(nc, in_maps, core_ids=list(range(8)))
    out = np.zeros((B, S, E), np.float32)
    for c in range(8):
        out[c // 4] += res.results[c]["out"].T
    out += bo[None, None, :]
    return out
